# revision 16
# baseline (speedup 1.0000x reference)
"""Trainium2 Bass kernel for nn_GatedAtomUpdate (gnn_message_passing).

Strategy (no collectives needed):
  - Host sorts bonds by receiver atom and buckets them into 8 contiguous
    atom ranges (12500 atoms/core). Each core computes the gated MLP for
    its own bonds and segment-sums locally into its own atom slice; the
    host concatenates the 8 output slices. No all-reduce.
  - Bonds are packed into 128-bond tiles, each tile assigned to a single
    64-atom block (pad bonds get an all-zero one-hot row).
  - The ACT (scalar) engine is the bottleneck: the math needs 4
    activation evals per bond-feature (silu h1, silu g1, silu h2,
    sigmoid g2) and ACT runs 1 elem/lane/cycle + ~222cyc/instr overhead.
    Three changes vs the 453us baseline:
      1. The gate sigmoid moves OFF ACT onto the DVE as a calibrated
         clamped affine: sigma(x) ~= clamp(a*x + 0.5, 0, 1). Two
         tensor_scalar ops (one reading PSUM directly).
      2. The one-hot scatter matrix S8 is precomputed on the HOST and
         DMA'd in (DMA has ~3x headroom), removing the per-tile
         IS_EQ build from DVE.
      3. L2 outputs for a PAIR of batches accumulate into one PSUM tile
         [128, 16, 128] so the h2-silu runs as one FD=1024 ACT
         instruction per pair, halving its per-instr overhead.
  - Device pipeline per 1024-bond batch:
      L1:  psum1[128(h|g), 1024] = [W1|G1]^T @ x^T   (two K=64 row-group
           MMs, row-packed at PE tile rows 0/64 -> stream concurrently)
      ACT: act1 = silu(psum1 + [b1;g1])              (one FD=1024 instr)
      L2:  psum2pair[128 bonds, 16, 128]: per-tile MM, stationary=act1
           tile, moving=blockdiag(W2,G2) -> bond-major [h2pre | g2pre]
      per PAIR of batches:
      ACT: h2 = silu(h2pre)                          (one FD=1024 instr)
      DVE: u    = a*g2pre + 0.5   (tensor_scalar, PSUM src)
           u    = clamp(u, 0, 1)  (tensor_scalar, 4x)
           msg  = h2 * u          (tensor_tensor, 2x)
      SEG: PE matmul msg^T @ S8 accumulates [64 feat, 64 atom] PSUM
           blocks; on block close DVE adds the atom_features slice.
"""

import math

import numpy as np
import ml_dtypes

import bass_rust
import concourse.bass as bass
import concourse.mybir as mybir
import concourse.tile as tile
from concourse.bass_utils import run_bass_kernel_spmd


def _ensure_axon_hooks():
    """bass_utils imports antenv.axon_hooks when tracing is requested (e.g.
    BASS_TRACE in the environment). Some images lack that module; install a
    graceful fallback so the kernel still runs (tracing is skipped when the
    injected libaxon has no profile symbols)."""
    try:
        import antenv.axon_hooks  # noqa: F401
        return
    except Exception:
        pass
    try:
        import sys
        import types

        import antenv
    except Exception:
        return
    mod = types.ModuleType("antenv.axon_hooks")
    _box = [None]
    mod.set_axon_ntff_profile_hook = lambda h: _box.__setitem__(0, h)
    mod.get_axon_ntff_profile_hook = lambda: _box[0]
    try:
        import contextlib
        import ctypes

        lib = ctypes.CDLL("/opt/axon/libaxon_pjrt.so")
        if hasattr(lib, "axon_start_nrt_profile"):
            lib.axon_start_nrt_profile.argtypes = [
                ctypes.POINTER(ctypes.c_int64),
                ctypes.c_size_t,
            ]
            lib.axon_start_nrt_profile.restype = ctypes.c_int64
            lib.axon_stop_nrt_profile.argtypes = [ctypes.c_char_p]
            lib.axon_stop_nrt_profile.restype = ctypes.c_int64

            @contextlib.contextmanager
            def _hook(output_dir, device_ids):
                import jax

                jax.devices()
                if device_ids:
                    ids = (ctypes.c_int64 * len(device_ids))(*device_ids)
                    rc = lib.axon_start_nrt_profile(ids, len(device_ids))
                else:
                    rc = lib.axon_start_nrt_profile(None, 0)
                if rc != 0:
                    raise RuntimeError(f"axon_start_nrt_profile rc={rc}")
                try:
                    yield
                finally:
                    lib.axon_stop_nrt_profile(str(output_dir).encode())

            _box[0] = _hook
    except Exception:
        pass
    sys.modules["antenv.axon_hooks"] = mod
    antenv.axon_hooks = mod


_ensure_axon_hooks()

# ---------------------------------------------------------------- constants
N_CORES = 8
D = 64
N_ATOMS = 100000
N_BONDS = 1500000
NA_CORE = N_ATOMS // N_CORES          # 12500
BLK = 64                              # atoms per block (one-hot width)
NBLK = math.ceil(NA_CORE / BLK)       # 196 blocks/core
NA_PAD = NBLK * BLK                   # 12544
TPB = 128                             # bonds per tile
L2B = 8                               # tiles per batch (1024 bonds)
CHUNK_B = 16                          # batches per xt/s8 DMA chunk
ATOM_CHUNKS = 8                       # atom-feature DMA split (ramp overlap)
OUT_CHUNKS = 16                       # output DMA split (tail overlap)

_BF16 = mybir.dt.bfloat16
_F32 = mybir.dt.float32


# --------------------------------------------- custom DVE op: 2*sigmoid(x)
# f2(x) = 1 + s*(C2 - |s|),  s = clamp(C0*x, -C1, C1)  ~=  2*sigmoid(x)
# (piecewise-quadratic, monotone, saturates at ~0/2 for all x; max err
# ~0.013 in sigma units). The doubled output is undone in the final
# atom add (affine_then_add with scale=0.5), so the gate multiply stays
# a stock 2x tensor_tensor. Constants minimize max |f2/2 - sigmoid|.
GATE_C0 = 0.26600
GATE_C1 = 0.92000
GATE_C2 = 1.97570


def _register_gate_op():
    import concourse.dve_ops as dve_ops
    from concourse.dve_spec import (
        Spec, Src0, C0, C1, C2, Zero, One, lower, maxx, minn,
        _has_src1,
    )
    from concourse.dve_uop import DveOpSpec

    name = "GATE2_SIG_ANT"
    for op in dve_ops.OPS:
        if op.name == name:
            return op

    t = Src0 * C0
    s = minn(maxx(t, Zero - C1), C1)
    a = maxx(s, Zero - s)
    body = One + s * (C2 - a)

    def _ref(in0, in1, s0, s1, imm2):
        ss = np.clip(np.asarray(in0, np.float32) * s0, -s1, s1)
        return (1.0 + ss * (imm2 - np.abs(ss))).astype(np.float32)

    spec = Spec(body=body, reference=_ref)
    row = dve_ops._CUSTOM_DVE_ROW_BASE + len(dve_ops.OPS)
    shas = {}
    for ver in ("v3", "v4"):
        tmp = DveOpSpec(
            name=name, opcode=row, uops=lower(spec, ver=ver),
            rd1_en=_has_src1(spec),
        )
        shas[ver] = tmp.sha(ver)
    gate = dve_ops.DveOp(name, spec, subdim=False, uops_sha=shas)
    dve_ops.OPS.append(gate)
    dve_ops._SUB_OPCODE_FOR_NAME[name] = row
    dve_ops.CUSTOM_DVE_SPECS[name] = spec
    return gate


GATE2_OP = _register_gate_op()


# ------------------------------------------------------- walrus workaround
def _patched_drain_and_barrier(self, tick_clock, wait_clock):
    """This walrus build accepts at most ONE sync-wait on TPB_CTRL-class
    instructions (Drain/NoOp), but TileContext's exit drain attaches one
    wait per DMA completion lane. Emit the waits on single-wait NOPs on
    the same engine first (program order gives the same guarantee), leave
    the drain bare, and reset semaphores one at a time."""
    nc = self.nc
    gc = tick_clock.global_clock
    ticks = list(gc)
    n = len(ticks)
    for proc, t in enumerate(ticks):
        if t > 0:
            vcp = bass_rust.VectorClock([t if j == proc else 0 for j in range(n)])
            nop = nc.sync.nop()
            wait_clock.add_sem_waits(nop.ins, tile.ScopedClock({None: vcp}))
    nc.sync.drain()
    nc.all_engine_barrier()
    assert self.sems is not None
    popped = nc._tile_sem_poison_stack.pop()
    assert popped is self._sem_poison
    for s in list(self.sems.allocated().values()):
        nc.clear_and_free_semaphores([s])
    nc.all_engine_barrier()


tile.TileContext._drain_and_barrier = _patched_drain_and_barrier


def _split_multi_waits(bir):
    """This walrus build rejects >1 sync-wait on an instruction ('Too many
    sync wait commands'). Move extra waits onto fresh single-wait NoOps
    inserted immediately before the instruction on the same engine —
    program order on the engine's sequencer preserves semantics."""
    n_new = 0
    for fn in bir.get("functions", []):
        for bb in fn.get("blocks", []):
            insts = bb.get("instructions", [])
            out = []
            for inst in insts:
                si = inst.get("sync_info") or {}
                ow = si.get("on_wait") or []
                if len(ow) > 1:
                    for i, w in enumerate(ow[:-1]):
                        out.append({
                            "name": f"{inst['name']}_sw{i}",
                            "opcode": "NoOp",
                            "engine": inst["engine"],
                            "ins": [],
                            "outs": [],
                            "sync_info": {"on_update": [], "on_wait": [w]},
                            "debug": inst.get("debug", 0),
                        })
                        n_new += 1
                    si["on_wait"] = [ow[-1]]
                out.append(inst)
            bb["instructions"] = out
    return n_new


_orig_to_json_bytes = bass.Bass.to_json_bytes


def _to_json_bytes_patched(self, *args, **kwargs):
    import json as _json
    raw = _orig_to_json_bytes(self, *args, **kwargs)
    bir = _json.loads(raw)
    n = _split_multi_waits(bir)
    if n == 0:
        return raw
    return _json.dumps(bir).encode()


bass.Bass.to_json_bytes = _to_json_bytes_patched


# ------------------------------------------------------------ host sharding
def _plan_and_pack(atom_features, bond_features, bond_atom_indices):
    """Sort bonds by receiver, bucket to cores/blocks, build a tile schedule
    shared by all cores (SPMD: one instruction stream), and pack per-core
    input arrays (bond features + host-built one-hot scatter tiles)."""
    recv = bond_atom_indices[:, 1].astype(np.int64)
    order = np.argsort(recv, kind="stable")
    sorted_recv = recv[order]
    core_edges = np.searchsorted(sorted_recv, np.arange(N_CORES + 1) * NA_CORE)

    # per-core, per-block bond counts
    cnt = np.zeros((N_CORES, NBLK), dtype=np.int64)
    locals_ = []
    for c in range(N_CORES):
        lo, hi = core_edges[c], core_edges[c + 1]
        local = sorted_recv[lo:hi] - c * NA_CORE
        locals_.append(local)
        cnt[c] = np.bincount(local // BLK, minlength=NBLK)

    # shared tile schedule: tiles per block (>=1 so every block is written);
    # total padded to an even number of batches (paired psum2 tiles)
    T = np.maximum(1, -(-cnt.max(axis=0) // TPB))
    ntiles = int(T.sum())
    pad_tiles = (-ntiles) % (2 * L2B)
    T[-1] += pad_tiles
    ntiles += pad_tiles
    tstart = np.concatenate([[0], np.cumsum(T)[:-1]]).astype(np.int64)

    # block id for every tile, in order
    tile_block = np.repeat(np.arange(NBLK), T)

    xt_list, s8_list = [], []
    nslots = ntiles * TPB
    arange64 = np.arange(BLK, dtype=np.float32)
    for c in range(N_CORES):
        local = locals_[c]
        blk = local // BLK
        block_off = np.concatenate([[0], np.cumsum(cnt[c])[:-1]])
        off_in_block = np.arange(local.shape[0]) - block_off[blk]
        slot = tstart[blk] * TPB + off_in_block

        gather = np.full(nslots, -1, dtype=np.int64)
        gather[slot] = order[core_edges[c]:core_edges[c + 1]]
        rel = np.full(nslots, 255.0, dtype=np.float32)
        rel[slot] = (local - blk * BLK).astype(np.float32)

        x_slot = np.zeros((nslots, D), dtype=np.float32)
        valid = gather >= 0
        x_slot[valid] = bond_features[gather[valid]]

        # pack into 128 partitions: row h*64+f, col m2*512+j holds
        # feature f of bond slot m2*1024 + h*512 + j  (h = 0/1 selects the
        # PE row-group the L1 matmul for that half streams from)
        nb2 = ntiles // L2B
        xs = x_slot.reshape(nb2, 2, 512, D)
        xt = np.ascontiguousarray(
            xs.transpose(1, 3, 0, 2).reshape(2 * D, nb2 * 512)
        ).astype(ml_dtypes.bfloat16)
        xt_list.append(xt)

        # host-built one-hot: s8[p, t*64 + a] = (rel[t*128+p] == a)
        rel2 = rel.reshape(ntiles, TPB).T                     # [128, ntiles]
        s8 = (rel2[:, :, None] == arange64[None, None, :])
        s8 = np.ascontiguousarray(
            s8.reshape(TPB, ntiles * BLK)
        ).astype(ml_dtypes.bfloat16)
        s8_list.append(s8)

    atomT_list = []
    for c in range(N_CORES):
        ap = np.zeros((NA_PAD, D), dtype=np.float32)
        ap[:NA_CORE] = atom_features[c * NA_CORE:(c + 1) * NA_CORE]
        atomT_list.append(np.ascontiguousarray(ap.T))

    return ntiles, tile_block, xt_list, s8_list, atomT_list


def _pack_weights(W1, G1, W2, G2, b1, g1, b2, g2):
    wg1_row = np.concatenate([W1, G1], axis=1)              # [64, 128]
    wg1 = np.concatenate([wg1_row, wg1_row], axis=0)        # [128, 128]
    wg2 = np.zeros((2 * D, 2 * D), dtype=np.float32)
    wg2[:D, :D] = W2
    wg2[D:, D:] = G2
    b1g1 = np.concatenate([b1, g1]).reshape(2 * D, 1).astype(np.float32)
    b2g2 = np.concatenate([b2, g2]).reshape(1, 2 * D)
    return (
        wg1.astype(ml_dtypes.bfloat16),
        wg2.astype(ml_dtypes.bfloat16),
        b1g1,
        b2g2.astype(ml_dtypes.bfloat16),
    )


# ------------------------------------------------------------- device kernel
def _build_nc(ntiles, tile_block, has_bias1, has_bias2):
    nb2 = ntiles // L2B
    assert nb2 % 2 == 0
    nc = bass.Bass()

    xt_d = nc.dram_tensor("xt", [2 * D, nb2 * 512], _BF16, kind="ExternalInput")
    s8_d = nc.dram_tensor("s8", [TPB, ntiles * BLK], _BF16, kind="ExternalInput")
    atomT_d = nc.dram_tensor("atomT", [D, NA_PAD], _F32, kind="ExternalInput")
    wg1_d = nc.dram_tensor("wg1", [2 * D, 2 * D], _BF16, kind="ExternalInput")
    wg2_d = nc.dram_tensor("wg2", [2 * D, 2 * D], _BF16, kind="ExternalInput")
    b1g1_d = nc.dram_tensor("b1g1", [2 * D, 1], _F32, kind="ExternalInput")
    b2g2_d = nc.dram_tensor("b2g2", [1, 2 * D], _BF16, kind="ExternalInput")
    out_d = nc.dram_tensor("out", [D, NA_PAD], _F32, kind="ExternalOutput")

    AF = mybir.ActivationFunctionType

    # first/last tile flags per block
    first_of_block = np.zeros(ntiles, dtype=bool)
    last_of_block = np.zeros(ntiles, dtype=bool)
    prev = -1
    for t in range(ntiles):
        b = tile_block[t]
        if b != prev:
            first_of_block[t] = True
            if t > 0:
                last_of_block[t - 1] = True
            prev = b
    last_of_block[ntiles - 1] = True

    with tile.TileContext(nc) as tc:
        with (
            tc.tile_pool(name="singles", bufs=1) as singles,
            tc.tile_pool(name="xtp", bufs=2) as xtp,
            tc.tile_pool(name="s8p", bufs=2) as s8p,
            tc.tile_pool(name="actp", bufs=3) as actp,
            tc.tile_pool(name="l2p", bufs=2) as l2p,
            # PSUM budget (8 banks): psum1 [128,1024]f32 x2 bufs = 4,
            # psum2 [128,8,128]f32 x1 = 2, pseg x2 = 2. psum1 double-
            # buffering lets PE run L1(m+1) during act1(m) so the ACT
            # stream never stalls on the L1 latency.
            tc.tile_pool(name="psum1p", bufs=2, space="PSUM") as psum1p,
            tc.tile_pool(name="psum2p", bufs=1, space="PSUM") as psum2p,
            tc.tile_pool(name="psegp", bufs=2, space="PSUM") as psegp,
        ):
            # Each dma_start doorbell costs ~650ns of serial Sync-queue issue
            # time; the first L1 matmul needs only wg1 + the first xt
            # mini-chunk, so only wg1's DMA is emitted here and everything
            # else is deferred into the m2==0 branch after the xt mini-chunk.
            wg1_sb = singles.tile([2 * D, 2 * D], _BF16)
            nc.sync.dma_start(out=wg1_sb[:], in_=wg1_d[:, :])
            wg2_sb = singles.tile([2 * D, 2 * D], _BF16)
            b1g1_sb = singles.tile([2 * D, 1], _F32)
            b2g2_sb = singles.tile([1, 2 * D], _BF16)
            ones_sb = singles.tile([1, 2 * D], _BF16)
            nc.vector.memset(ones_sb[:], 1.0)
            atom_sb = singles.tile([D, NA_PAD], _F32)
            out_sb = singles.tile([D, NA_PAD], _F32)

            pseg_cur = None
            stash_act = {}    # m -> act1 handle awaiting L2
            stash_post = {}   # m -> psum2 tile awaiting act+mult
            stash = {}        # m -> msg handle awaiting segment acc

            def emit_l2(j):
                """L2 matmuls for batch j: per-tile stationary=act1 slice,
                moving=blockdiag(W2|G2)."""
                a = stash_act.pop(j)
                psum2 = psum2p.tile([TPB, L2B, 2 * D], _F32, tag="psum2")
                for tt in range(L2B):
                    sl = tt * TPB
                    nc.tensor.matmul(
                        psum2[:, tt, :], a[:, sl:sl + TPB], wg2_sb[:, :],
                        start=True, stop=not has_bias2,
                    )
                    if has_bias2:
                        nc.tensor.matmul(
                            psum2[:, tt, :], ones_sb[0:1, :],
                            b2g2_sb[0:1, :],
                            start=False, stop=True,
                        )
                stash_post[j] = psum2

            def emit_post(j):
                """Post-L2 stage: silu on ACT for h2pre, one fused
                custom-DVE op for ~2*sigmoid(g2pre), then the gate
                multiply (msg = 2x true message; the final atom add
                rescales by 0.5). Emitted late so inputs are ready."""
                psum2_j = stash_post.pop(j)
                h2 = l2p.tile([TPB, L2B, D], _BF16, tag="h2")
                nc.scalar.activation(h2[:], psum2_j[:, :, 0:D], AF.Silu)
                f2 = l2p.tile([TPB, L2B, D], _BF16, tag="f2")
                nc.vector._custom_dve(
                    GATE2_OP, out=f2[:], in0=psum2_j[:, :, D:2 * D],
                    s0=GATE_C0, s1=GATE_C1, imm2=GATE_C2,
                )
                msg = l2p.tile([TPB, L2B, D], _BF16, tag="msg")
                nc.vector.tensor_tensor(msg[:], h2[:], f2[:], mybir.AluOpType.mult)
                stash[j] = msg

            # blocks close in tile order; stream the output back to HBM in
            # chunks as soon as the last block of each chunk is done so the
            # final transfer isn't serialized after the last matmul
            out_edges = [
                (k * NBLK) // OUT_CHUNKS for k in range(1, OUT_CHUNKS + 1)
            ]

            def emit_seg(j):
                nonlocal pseg_cur
                msg_q = stash.pop(j)
                t0q = j * L2B
                # a batch's tiles live in one 16-batch s8 chunk; pick the
                # kept handle that covers this batch's columns
                want = t0q * BLK
                s8_sb, s8_c0 = next(
                    (sb, c0) for (sb, c0) in s8_chunk
                    if c0 <= want < c0 + CHUNK_B * 512
                )
                for tt in range(L2B):
                    t_glob = t0q + tt
                    b = int(tile_block[t_glob])
                    if first_of_block[t_glob]:
                        pseg_cur = psegp.tile([D, BLK], _F32, tag="pseg")
                    so = t_glob * BLK - s8_c0
                    nc.tensor.matmul(
                        pseg_cur[:, :], msg_q[:, tt, :],
                        s8_sb[:, so:so + BLK],
                        start=bool(first_of_block[t_glob]),
                        stop=bool(last_of_block[t_glob]),
                    )
                    if last_of_block[t_glob]:
                        # messages were doubled by the gate op; rescale here
                        nc.vector.affine_then_add(
                            out_sb[:, b * BLK:(b + 1) * BLK],
                            pseg_cur[:, :],
                            atom_sb[:, b * BLK:(b + 1) * BLK],
                            scale=0.5, bias=0.0,
                        )
                        if b + 1 in out_edges:
                            lo = out_edges[out_edges.index(b + 1) - 1] * BLK \
                                if out_edges.index(b + 1) > 0 else 0
                            nc.sync.dma_start(
                                out=out_d[:, lo:(b + 1) * BLK],
                                in_=out_sb[:, lo:(b + 1) * BLK],
                            )

            # s8 chunk bookkeeping: emit_seg(q) runs ~5 batches behind the
            # xt chunk loop, so the s8 chunk DMA is issued on the same
            # 16-batch cadence but the chunk consumed by emit_seg is the
            # PREVIOUS one for the first 5 batches of each chunk window.
            # Keep the last two chunk handles.
            s8_chunk = []  # [(tile, col0)], newest first

            for m2 in range(nb2):
                # ---- xt + s8 chunk DMA (every CHUNK_B batches); the first
                # chunk is split so compute starts after 512KB instead of
                # 2MB; s8 is only needed by emit_seg (runs ~5 batches late)
                if m2 % CHUNK_B == 0:
                    w = min(CHUNK_B, nb2 - m2) * 512
                    xt_sb = xtp.tile([2 * D, CHUNK_B * 512], _BF16, tag="xt")
                    if m2 == 0:
                        nc.sync.dma_start(
                            out=xt_sb[:, 0:2048], in_=xt_d[:, 0:2048]
                        )
                        nc.sync.dma_start(out=b1g1_sb[:], in_=b1g1_d[:, :])
                        nc.sync.dma_start(out=wg2_sb[:], in_=wg2_d[:, :])
                        nc.sync.dma_start(out=b2g2_sb[:], in_=b2g2_d[:, :])
                        ahi0 = NA_PAD // ATOM_CHUNKS
                        nc.sync.dma_start(
                            out=atom_sb[:, 0:ahi0], in_=atomT_d[:, 0:ahi0]
                        )
                        nc.sync.dma_start(
                            out=xt_sb[:, 2048:4096], in_=xt_d[:, 2048:4096]
                        )
                        nc.sync.dma_start(
                            out=xt_sb[:, 4096:w], in_=xt_d[:, 4096:w]
                        )
                    else:
                        nc.sync.dma_start(
                            out=xt_sb[:, :w],
                            in_=xt_d[:, m2 * 512: m2 * 512 + w],
                        )
                    s8_sb = s8p.tile([TPB, CHUNK_B * 512], _BF16, tag="s8")
                    c0 = m2 * L2B * BLK
                    ws = min(CHUNK_B, nb2 - m2) * 512
                    nc.sync.dma_start(
                        out=s8_sb[:, :ws], in_=s8_d[:, c0:c0 + ws]
                    )
                    s8_chunk.insert(0, (s8_sb, c0))
                    del s8_chunk[2:]

                # ---- L1: two K=64 row-packed MMs (PE rows 0-63 / 64-127
                # stream concurrently), one FD=1024 silu over both halves
                co = (m2 % CHUNK_B) * 512
                psum1 = psum1p.tile([2 * D, 1024], _F32, tag="psum1")
                nc.tensor.matmul(
                    psum1[:, 0:512], wg1_sb[0:D, :], xt_sb[0:D, co:co + 512],
                    start=True, stop=True,
                )
                nc.tensor.matmul(
                    psum1[:, 512:1024], wg1_sb[D:2 * D, :],
                    xt_sb[D:2 * D, co:co + 512],
                    start=True, stop=True,
                )
                act1 = actp.tile([2 * D, 1024], _BF16, tag="act1")
                if has_bias1:
                    nc.scalar.activation(
                        act1[:], psum1[:], AF.Silu, bias=b1g1_sb[:, 0:1],
                        scale=1.0,
                    )
                else:
                    nc.scalar.activation(act1[:], psum1[:], AF.Silu)
                stash_act[m2] = act1

                # post-L2 stage for batch m2-2, right after act1 in the
                # ACT queue; its psum2 read releases the single psum2
                # buffer for L2(m2-1) just below
                if m2 >= 2:
                    emit_post(m2 - 2)

                # ---- L2 for batch m2-1 (act1 one iteration old, so the
                # act1->L2->h2 chain spans iterations instead of serializing
                # inside one period)
                if m2 >= 1:
                    emit_l2(m2 - 1)

                # ---- atom features arrive in chunks during the first
                # iterations instead of one serial 3.2MB DMA before compute
                if 1 <= m2 < ATOM_CHUNKS:
                    alo = (m2 * NA_PAD) // ATOM_CHUNKS
                    ahi = ((m2 + 1) * NA_PAD) // ATOM_CHUNKS
                    nc.sync.dma_start(
                        out=atom_sb[:, alo:ahi], in_=atomT_d[:, alo:ahi]
                    )

                # ---- segment accumulation for batch m2-4
                if m2 >= 4:
                    emit_seg(m2 - 4)

            emit_l2(nb2 - 1)
            for j in range(nb2 - 2, nb2):
                emit_post(j)
            for j in range(nb2 - 4, nb2):
                emit_seg(j)

    # run_bass_kernel_spmd's serialization path skips the Bacc compile step
    # that fills in the 64-byte ISA payload of InstCustomDveAnt; without
    # this, walrus codegen fails with "ISA wrong length".
    mybir.codegen_inst_isa_subclasses(nc)
    return nc


# ----------------------------------------------------------------- kernel()
LAST_EXEC_NS = None
LAST_RESULT = None


def kernel(**inputs):
    atom_features = np.asarray(inputs["atom_features"], dtype=np.float32)
    bond_features = np.asarray(inputs["bond_features"], dtype=np.float32)
    bond_atom_indices = np.asarray(inputs["bond_atom_indices"])
    W1 = np.asarray(inputs["W1"], dtype=np.float32)
    W2 = np.asarray(inputs["W2"], dtype=np.float32)
    G1 = np.asarray(inputs["G1"], dtype=np.float32)
    G2 = np.asarray(inputs["G2"], dtype=np.float32)
    b1 = np.asarray(inputs["b1"], dtype=np.float32)
    b2 = np.asarray(inputs["b2"], dtype=np.float32)
    g1 = np.asarray(inputs["g1"], dtype=np.float32)
    g2 = np.asarray(inputs["g2"], dtype=np.float32)

    ntiles, tile_block, xt_list, s8_list, atomT_list = _plan_and_pack(
        atom_features, bond_features, bond_atom_indices
    )
    wg1, wg2, b1g1, b2g2 = _pack_weights(W1, G1, W2, G2, b1, g1, b2, g2)
    has_bias1 = not (np.all(b1 == 0.0) and np.all(g1 == 0.0))
    has_bias2 = not (np.all(b2 == 0.0) and np.all(g2 == 0.0))

    nc = _build_nc(ntiles, tile_block, has_bias1, has_bias2)

    in_maps = []
    for c in range(N_CORES):
        in_maps.append({
            "xt": xt_list[c],
            "s8": s8_list[c],
            "atomT": atomT_list[c],
            "wg1": wg1,
            "wg2": wg2,
            "b1g1": b1g1,
            "b2g2": b2g2,
        })

    import os as _os
    _trace = bool(int(_os.environ.get("KERNEL_TRACE", "0")))
    res = run_bass_kernel_spmd(nc, in_maps, core_ids=list(range(N_CORES)), trace=_trace)
    global LAST_EXEC_NS, LAST_RESULT
    LAST_EXEC_NS = res.exec_time_ns
    LAST_RESULT = res

    out = np.empty((N_ATOMS, D), dtype=np.float32)
    for c in range(N_CORES):
        out[c * NA_CORE:(c + 1) * NA_CORE] = res.results[c]["out"][:, :NA_CORE].T
    return out


# revision 20
# speedup vs baseline: 1.0593x; 1.0593x over previous
"""Trainium2 Bass kernel for nn_GatedAtomUpdate (gnn_message_passing).

Strategy (no collectives needed):
  - Host sorts bonds by receiver atom and buckets them into 8 contiguous
    atom ranges (12500 atoms/core). Each core computes the gated MLP for
    its own bonds and segment-sums locally into its own atom slice; the
    host concatenates the 8 output slices. No all-reduce.
  - Bonds are packed into 128-bond tiles, each tile assigned to a single
    64-atom block (pad bonds get an all-zero one-hot row).
  - The ACT (scalar) engine is the bottleneck: the math needs 4
    activation evals per bond-feature (silu h1, silu g1, silu h2,
    sigmoid g2) and ACT runs 1 elem/lane/cycle + ~222cyc/instr overhead.
    Three changes vs the 453us baseline:
      1. The gate sigmoid moves OFF ACT onto the DVE as a calibrated
         clamped affine: sigma(x) ~= clamp(a*x + 0.5, 0, 1). Two
         tensor_scalar ops (one reading PSUM directly).
      2. The one-hot scatter matrix S8 is precomputed on the HOST and
         DMA'd in (DMA has ~3x headroom), removing the per-tile
         IS_EQ build from DVE.
      3. L2 outputs for a PAIR of batches accumulate into one PSUM tile
         [128, 16, 128] so the h2-silu runs as one FD=1024 ACT
         instruction per pair, halving its per-instr overhead.
  - Device pipeline per 1024-bond batch:
      L1:  psum1[128(h|g), 1024] = [W1|G1]^T @ x^T   (two K=64 row-group
           MMs, row-packed at PE tile rows 0/64 -> stream concurrently)
      ACT: act1 = silu(psum1 + [b1;g1])              (one FD=1024 instr)
      L2:  psum2pair[128 bonds, 16, 128]: per-tile MM, stationary=act1
           tile, moving=blockdiag(W2,G2) -> bond-major [h2pre | g2pre]
      per PAIR of batches:
      ACT: h2 = silu(h2pre)                          (one FD=1024 instr)
      DVE: u    = a*g2pre + 0.5   (tensor_scalar, PSUM src)
           u    = clamp(u, 0, 1)  (tensor_scalar, 4x)
           msg  = h2 * u          (tensor_tensor, 2x)
      SEG: PE matmul msg^T @ S8 accumulates [64 feat, 64 atom] PSUM
           blocks; on block close DVE adds the atom_features slice.
"""

import math

import numpy as np
import ml_dtypes

import bass_rust
import concourse.bass as bass
import concourse.mybir as mybir
import concourse.tile as tile
from concourse.bass_utils import run_bass_kernel_spmd


def _ensure_axon_hooks():
    """bass_utils imports antenv.axon_hooks when tracing is requested (e.g.
    BASS_TRACE in the environment). Some images lack that module; install a
    graceful fallback so the kernel still runs (tracing is skipped when the
    injected libaxon has no profile symbols)."""
    try:
        import antenv.axon_hooks  # noqa: F401
        return
    except Exception:
        pass
    try:
        import sys
        import types

        import antenv
    except Exception:
        return
    mod = types.ModuleType("antenv.axon_hooks")
    _box = [None]
    mod.set_axon_ntff_profile_hook = lambda h: _box.__setitem__(0, h)
    mod.get_axon_ntff_profile_hook = lambda: _box[0]
    try:
        import contextlib
        import ctypes

        lib = ctypes.CDLL("/opt/axon/libaxon_pjrt.so")
        if hasattr(lib, "axon_start_nrt_profile"):
            lib.axon_start_nrt_profile.argtypes = [
                ctypes.POINTER(ctypes.c_int64),
                ctypes.c_size_t,
            ]
            lib.axon_start_nrt_profile.restype = ctypes.c_int64
            lib.axon_stop_nrt_profile.argtypes = [ctypes.c_char_p]
            lib.axon_stop_nrt_profile.restype = ctypes.c_int64

            @contextlib.contextmanager
            def _hook(output_dir, device_ids):
                import jax

                jax.devices()
                if device_ids:
                    ids = (ctypes.c_int64 * len(device_ids))(*device_ids)
                    rc = lib.axon_start_nrt_profile(ids, len(device_ids))
                else:
                    rc = lib.axon_start_nrt_profile(None, 0)
                if rc != 0:
                    raise RuntimeError(f"axon_start_nrt_profile rc={rc}")
                try:
                    yield
                finally:
                    lib.axon_stop_nrt_profile(str(output_dir).encode())

            _box[0] = _hook
    except Exception:
        pass
    sys.modules["antenv.axon_hooks"] = mod
    antenv.axon_hooks = mod


_ensure_axon_hooks()

# ---------------------------------------------------------------- constants
N_CORES = 8
D = 64
N_ATOMS = 100000
N_BONDS = 1500000
NA_CORE = N_ATOMS // N_CORES          # 12500
BLK = 64                              # atoms per block (one-hot width)
NBLK = math.ceil(NA_CORE / BLK)       # 196 blocks/core
NA_PAD = NBLK * BLK                   # 12544
TPB = 128                             # bonds per tile
L2B = 8                               # tiles per batch (1024 bonds)
CHUNK_B = 16                          # batches per xt/s8 DMA chunk
ATOM_CHUNKS = 8                       # atom-feature DMA split (ramp overlap)
OUT_CHUNKS = 16                       # output DMA split (tail overlap)

_BF16 = mybir.dt.bfloat16
_F32 = mybir.dt.float32


# --------------------------------------------- custom DVE op: 2*sigmoid(x)
# f2(x) = 1 + s*(C2 - |s|),  s = clamp(C0*x, -C1, C1)  ~=  2*sigmoid(x)
# (piecewise-quadratic, monotone, saturates at ~0/2 for all x; max err
# ~0.013 in sigma units). The doubled output is undone in the final
# atom add (affine_then_add with scale=0.5), so the gate multiply stays
# a stock 2x tensor_tensor. Constants minimize max |f2/2 - sigmoid|.
GATE_C0 = 0.26600
GATE_C1 = 0.92000
GATE_C2 = 1.97570


def _register_gate_op():
    import concourse.dve_ops as dve_ops
    from concourse.dve_spec import (
        Spec, Src0, C0, C1, C2, Zero, One, lower, maxx, minn,
        _has_src1,
    )
    from concourse.dve_uop import DveOpSpec

    name = "GATE2_SIG_ANT"
    for op in dve_ops.OPS:
        if op.name == name:
            return op

    t = Src0 * C0
    s = minn(maxx(t, Zero - C1), C1)
    a = maxx(s, Zero - s)
    body = One + s * (C2 - a)

    def _ref(in0, in1, s0, s1, imm2):
        ss = np.clip(np.asarray(in0, np.float32) * s0, -s1, s1)
        return (1.0 + ss * (imm2 - np.abs(ss))).astype(np.float32)

    spec = Spec(body=body, reference=_ref)
    row = dve_ops._CUSTOM_DVE_ROW_BASE + len(dve_ops.OPS)
    shas = {}
    for ver in ("v3", "v4"):
        tmp = DveOpSpec(
            name=name, opcode=row, uops=lower(spec, ver=ver),
            rd1_en=_has_src1(spec),
        )
        shas[ver] = tmp.sha(ver)
    gate = dve_ops.DveOp(name, spec, subdim=False, uops_sha=shas)
    dve_ops.OPS.append(gate)
    dve_ops._SUB_OPCODE_FOR_NAME[name] = row
    dve_ops.CUSTOM_DVE_SPECS[name] = spec
    return gate


GATE2_OP = _register_gate_op()


# ------------------------------------------------------- walrus workaround
def _patched_drain_and_barrier(self, tick_clock, wait_clock):
    """This walrus build accepts at most ONE sync-wait on TPB_CTRL-class
    instructions (Drain/NoOp), but TileContext's exit drain attaches one
    wait per DMA completion lane. Emit the waits on single-wait NOPs on
    the same engine first (program order gives the same guarantee), leave
    the drain bare, and reset semaphores one at a time."""
    nc = self.nc
    gc = tick_clock.global_clock
    ticks = list(gc)
    n = len(ticks)
    for proc, t in enumerate(ticks):
        if t > 0:
            vcp = bass_rust.VectorClock([t if j == proc else 0 for j in range(n)])
            nop = nc.sync.nop()
            wait_clock.add_sem_waits(nop.ins, tile.ScopedClock({None: vcp}))
    nc.sync.drain()
    nc.all_engine_barrier()
    assert self.sems is not None
    popped = nc._tile_sem_poison_stack.pop()
    assert popped is self._sem_poison
    for s in list(self.sems.allocated().values()):
        nc.clear_and_free_semaphores([s])
    nc.all_engine_barrier()


tile.TileContext._drain_and_barrier = _patched_drain_and_barrier


def _split_multi_waits(bir):
    """This walrus build rejects >1 sync-wait on an instruction ('Too many
    sync wait commands'). Move extra waits onto fresh single-wait NoOps
    inserted immediately before the instruction on the same engine —
    program order on the engine's sequencer preserves semantics."""
    n_new = 0
    for fn in bir.get("functions", []):
        for bb in fn.get("blocks", []):
            insts = bb.get("instructions", [])
            out = []
            for inst in insts:
                si = inst.get("sync_info") or {}
                ow = si.get("on_wait") or []
                if len(ow) > 1:
                    for i, w in enumerate(ow[:-1]):
                        out.append({
                            "name": f"{inst['name']}_sw{i}",
                            "opcode": "NoOp",
                            "engine": inst["engine"],
                            "ins": [],
                            "outs": [],
                            "sync_info": {"on_update": [], "on_wait": [w]},
                            "debug": inst.get("debug", 0),
                        })
                        n_new += 1
                    si["on_wait"] = [ow[-1]]
                out.append(inst)
            bb["instructions"] = out
    return n_new


_orig_to_json_bytes = bass.Bass.to_json_bytes


def _to_json_bytes_patched(self, *args, **kwargs):
    import json as _json
    raw = _orig_to_json_bytes(self, *args, **kwargs)
    bir = _json.loads(raw)
    n = _split_multi_waits(bir)
    if n == 0:
        return raw
    return _json.dumps(bir).encode()


bass.Bass.to_json_bytes = _to_json_bytes_patched


# ------------------------------------------------------------ host sharding
def _plan_and_pack(atom_features, bond_features, bond_atom_indices):
    """Sort bonds by receiver, bucket to cores/blocks, build a tile schedule
    shared by all cores (SPMD: one instruction stream), and pack per-core
    input arrays (bond features + host-built one-hot scatter tiles)."""
    recv = bond_atom_indices[:, 1].astype(np.int64)
    order = np.argsort(recv, kind="stable")
    sorted_recv = recv[order]
    core_edges = np.searchsorted(sorted_recv, np.arange(N_CORES + 1) * NA_CORE)

    # per-core, per-block bond counts
    cnt = np.zeros((N_CORES, NBLK), dtype=np.int64)
    locals_ = []
    for c in range(N_CORES):
        lo, hi = core_edges[c], core_edges[c + 1]
        local = sorted_recv[lo:hi] - c * NA_CORE
        locals_.append(local)
        cnt[c] = np.bincount(local // BLK, minlength=NBLK)

    # shared tile schedule: tiles per block (>=1 so every block is written);
    # total padded to an even number of batches (paired psum2 tiles)
    T = np.maximum(1, -(-cnt.max(axis=0) // TPB))
    ntiles = int(T.sum())
    pad_tiles = (-ntiles) % (2 * L2B)
    T[-1] += pad_tiles
    ntiles += pad_tiles
    tstart = np.concatenate([[0], np.cumsum(T)[:-1]]).astype(np.int64)

    # block id for every tile, in order
    tile_block = np.repeat(np.arange(NBLK), T)

    xt_list, s8_list = [], []
    nslots = ntiles * TPB
    arange64 = np.arange(BLK, dtype=np.float32)
    for c in range(N_CORES):
        local = locals_[c]
        blk = local // BLK
        block_off = np.concatenate([[0], np.cumsum(cnt[c])[:-1]])
        off_in_block = np.arange(local.shape[0]) - block_off[blk]
        slot = tstart[blk] * TPB + off_in_block

        gather = np.full(nslots, -1, dtype=np.int64)
        gather[slot] = order[core_edges[c]:core_edges[c + 1]]
        rel = np.full(nslots, 255.0, dtype=np.float32)
        rel[slot] = (local - blk * BLK).astype(np.float32)

        x_slot = np.zeros((nslots, D), dtype=np.float32)
        valid = gather >= 0
        x_slot[valid] = bond_features[gather[valid]]

        # pack into 128 partitions: row h*64+f, col m2*512+j holds
        # feature f of bond slot m2*1024 + h*512 + j  (h = 0/1 selects the
        # PE row-group the L1 matmul for that half streams from)
        nb2 = ntiles // L2B
        xs = x_slot.reshape(nb2, 2, 512, D)
        xt = np.ascontiguousarray(
            xs.transpose(1, 3, 0, 2).reshape(2 * D, nb2 * 512)
        ).astype(ml_dtypes.bfloat16)
        xt_list.append(xt)

        # host-built one-hot: s8[p, t*64 + a] = (rel[t*128+p] == a)
        rel2 = rel.reshape(ntiles, TPB).T                     # [128, ntiles]
        s8 = (rel2[:, :, None] == arange64[None, None, :])
        s8 = np.ascontiguousarray(
            s8.reshape(TPB, ntiles * BLK)
        ).astype(ml_dtypes.bfloat16)
        s8_list.append(s8)

    atomT_list = []
    for c in range(N_CORES):
        ap = np.zeros((NA_PAD, D), dtype=np.float32)
        ap[:NA_CORE] = atom_features[c * NA_CORE:(c + 1) * NA_CORE]
        atomT_list.append(np.ascontiguousarray(ap.T))

    return ntiles, tile_block, xt_list, s8_list, atomT_list


def _pack_weights(W1, G1, W2, G2, b1, g1, b2, g2):
    wg1_row = np.concatenate([W1, G1], axis=1)              # [64, 128]
    wg1 = np.concatenate([wg1_row, wg1_row], axis=0)        # [128, 128]
    wg2 = np.zeros((2 * D, 2 * D), dtype=np.float32)
    wg2[:D, :D] = W2
    wg2[D:, D:] = G2
    b1g1 = np.concatenate([b1, g1]).reshape(2 * D, 1).astype(np.float32)
    b2g2 = np.concatenate([b2, g2]).reshape(1, 2 * D)
    return (
        wg1.astype(ml_dtypes.bfloat16),
        wg2.astype(ml_dtypes.bfloat16),
        b1g1,
        b2g2.astype(ml_dtypes.bfloat16),
    )


# ------------------------------------------------------------- device kernel
def _build_nc(ntiles, tile_block, has_bias1, has_bias2):
    nb2 = ntiles // L2B
    assert nb2 % 2 == 0
    nc = bass.Bass()

    xt_d = nc.dram_tensor("xt", [2 * D, nb2 * 512], _BF16, kind="ExternalInput")
    s8_d = nc.dram_tensor("s8", [TPB, ntiles * BLK], _BF16, kind="ExternalInput")
    atomT_d = nc.dram_tensor("atomT", [D, NA_PAD], _F32, kind="ExternalInput")
    wg1_d = nc.dram_tensor("wg1", [2 * D, 2 * D], _BF16, kind="ExternalInput")
    wg2_d = nc.dram_tensor("wg2", [2 * D, 2 * D], _BF16, kind="ExternalInput")
    b1g1_d = nc.dram_tensor("b1g1", [2 * D, 1], _F32, kind="ExternalInput")
    b2g2_d = nc.dram_tensor("b2g2", [1, 2 * D], _BF16, kind="ExternalInput")
    out_d = nc.dram_tensor("out", [D, NA_PAD], _F32, kind="ExternalOutput")

    AF = mybir.ActivationFunctionType

    # first/last tile flags per block
    first_of_block = np.zeros(ntiles, dtype=bool)
    last_of_block = np.zeros(ntiles, dtype=bool)
    prev = -1
    for t in range(ntiles):
        b = tile_block[t]
        if b != prev:
            first_of_block[t] = True
            if t > 0:
                last_of_block[t - 1] = True
            prev = b
    last_of_block[ntiles - 1] = True

    with tile.TileContext(nc) as tc:
        with (
            tc.tile_pool(name="singles", bufs=1) as singles,
            tc.tile_pool(name="xtp", bufs=2) as xtp,
            tc.tile_pool(name="s8p", bufs=2) as s8p,
            tc.tile_pool(name="actp", bufs=3) as actp,
            tc.tile_pool(name="l2p", bufs=2) as l2p,
            # PSUM budget (8 banks): psum1 [128,1024]f32 x1 = 2, psum2
            # pair tile [128,16,128]f32 x1 = 4, pseg x2 = 2. The paired
            # psum2 gives the post-L2 stages a 2-batch window so the
            # PE<->DVE dependency chain has slack; the h2 silu is split
            # into two FD=512 instructions so both act1->act1 transitions
            # on ACT have filler work that hides the L1 matmul latency.
            tc.tile_pool(name="psum1p", bufs=1, space="PSUM") as psum1p,
            tc.tile_pool(name="psum2p", bufs=1, space="PSUM") as psum2p,
            tc.tile_pool(name="psegp", bufs=2, space="PSUM") as psegp,
        ):
            # Each dma_start doorbell costs ~650ns of serial Sync-queue issue
            # time; the first L1 matmul needs only wg1 + the first xt
            # mini-chunk, so only wg1's DMA is emitted here and everything
            # else is deferred into the m2==0 branch after the xt mini-chunk.
            wg1_sb = singles.tile([2 * D, 2 * D], _BF16)
            nc.sync.dma_start(out=wg1_sb[:], in_=wg1_d[:, :])
            wg2_sb = singles.tile([2 * D, 2 * D], _BF16)
            b1g1_sb = singles.tile([2 * D, 1], _F32)
            b2g2_sb = singles.tile([1, 2 * D], _BF16)
            ones_sb = singles.tile([1, 2 * D], _BF16)
            nc.vector.memset(ones_sb[:], 1.0)
            atom_sb = singles.tile([D, NA_PAD], _F32)
            out_sb = singles.tile([D, NA_PAD], _F32)

            pseg_cur = None
            stash_act = {}    # m -> act1 handle awaiting L2
            stash_ps2 = {}    # pair q -> psum2 pair tile being filled
            stash_post = {}   # pair q -> (psum2 pair tile, h2 tile)
            stash = {}        # pair q -> msg pair handle awaiting seg acc

            def emit_l2(j):
                """L2 matmuls for batch j into the pair-psum tile (halves
                j%2): per-tile stationary=act1 slice, moving=
                blockdiag(W2|G2)."""
                q, h = divmod(j, 2)
                a = stash_act.pop(j)
                if h == 0:
                    psum2_new = psum2p.tile(
                        [TPB, 2 * L2B, 2 * D], _F32, tag="psum2"
                    )
                    stash_ps2[q] = psum2_new
                psum2 = stash_ps2[q]
                for tt in range(L2B):
                    sl = tt * TPB
                    nc.tensor.matmul(
                        psum2[:, h * L2B + tt, :], a[:, sl:sl + TPB],
                        wg2_sb[:, :],
                        start=True, stop=not has_bias2,
                    )
                    if has_bias2:
                        nc.tensor.matmul(
                            psum2[:, h * L2B + tt, :], ones_sb[0:1, :],
                            b2g2_sb[0:1, :],
                            start=False, stop=True,
                        )
                if h == 1:
                    stash_post[q] = stash_ps2.pop(q)

            def emit_post_a(q):
                """First half of the pair's h2 silu (FD=512) — scheduled
                between the two act1's so PE's L1 latency is hidden."""
                psum2_q = stash_post[q]
                h2 = l2p.tile([TPB, 2 * L2B, D], _BF16, tag="h2")
                nc.scalar.activation(
                    h2[:, 0:L2B, :], psum2_q[:, 0:L2B, 0:D], AF.Silu
                )
                stash_post[q] = (psum2_q, h2)

            def emit_post_b(q):
                """Second h2 half + fused ~2*sigmoid gate (custom DVE) +
                gate multiply for the whole pair (msg = 2x true message;
                the final atom add rescales by 0.5)."""
                psum2_q, h2 = stash_post.pop(q)
                nc.scalar.activation(
                    h2[:, L2B:2 * L2B, :], psum2_q[:, L2B:2 * L2B, 0:D],
                    AF.Silu,
                )
                f2 = l2p.tile([TPB, 2 * L2B, D], _BF16, tag="f2")
                nc.vector._custom_dve(
                    GATE2_OP, out=f2[:], in0=psum2_q[:, :, D:2 * D],
                    s0=GATE_C0, s1=GATE_C1, imm2=GATE_C2,
                )
                msg = l2p.tile([TPB, 2 * L2B, D], _BF16, tag="msg")
                nc.vector.tensor_tensor(msg[:], h2[:], f2[:], mybir.AluOpType.mult)
                stash[q] = msg

            # blocks close in tile order; stream the output back to HBM in
            # chunks as soon as the last block of each chunk is done so the
            # final transfer isn't serialized after the last matmul
            out_edges = [
                (k * NBLK) // OUT_CHUNKS for k in range(1, OUT_CHUNKS + 1)
            ]

            def emit_seg(q):
                nonlocal pseg_cur
                msg_q = stash.pop(q)
                t0q = q * 2 * L2B
                # both batches of a pair live in one 16-batch s8 chunk;
                # pick the kept handle that covers this pair's columns
                want = t0q * BLK
                s8_sb, s8_c0 = next(
                    (sb, c0) for (sb, c0) in s8_chunk
                    if c0 <= want < c0 + CHUNK_B * 512
                )
                for tt in range(2 * L2B):
                    t_glob = t0q + tt
                    b = int(tile_block[t_glob])
                    if first_of_block[t_glob]:
                        pseg_cur = psegp.tile([D, BLK], _F32, tag="pseg")
                    so = t_glob * BLK - s8_c0
                    nc.tensor.matmul(
                        pseg_cur[:, :], msg_q[:, tt, :],
                        s8_sb[:, so:so + BLK],
                        start=bool(first_of_block[t_glob]),
                        stop=bool(last_of_block[t_glob]),
                    )
                    if last_of_block[t_glob]:
                        # messages were doubled by the gate op; rescale here
                        nc.vector.affine_then_add(
                            out_sb[:, b * BLK:(b + 1) * BLK],
                            pseg_cur[:, :],
                            atom_sb[:, b * BLK:(b + 1) * BLK],
                            scale=0.5, bias=0.0,
                        )
                        if b + 1 in out_edges:
                            lo = out_edges[out_edges.index(b + 1) - 1] * BLK \
                                if out_edges.index(b + 1) > 0 else 0
                            nc.sync.dma_start(
                                out=out_d[:, lo:(b + 1) * BLK],
                                in_=out_sb[:, lo:(b + 1) * BLK],
                            )

            # s8 chunk bookkeeping: emit_seg(q) runs ~5 batches behind the
            # xt chunk loop, so the s8 chunk DMA is issued on the same
            # 16-batch cadence but the chunk consumed by emit_seg is the
            # PREVIOUS one for the first 5 batches of each chunk window.
            # Keep the last two chunk handles.
            s8_chunk = []  # [(tile, col0)], newest first

            for m2 in range(nb2):
                # ---- xt + s8 chunk DMA (every CHUNK_B batches); the first
                # chunk is split so compute starts after 512KB instead of
                # 2MB; s8 is only needed by emit_seg (runs ~5 batches late)
                if m2 % CHUNK_B == 0:
                    w = min(CHUNK_B, nb2 - m2) * 512
                    xt_sb = xtp.tile([2 * D, CHUNK_B * 512], _BF16, tag="xt")
                    if m2 == 0:
                        nc.sync.dma_start(
                            out=xt_sb[:, 0:2048], in_=xt_d[:, 0:2048]
                        )
                        nc.sync.dma_start(out=b1g1_sb[:], in_=b1g1_d[:, :])
                        nc.sync.dma_start(out=wg2_sb[:], in_=wg2_d[:, :])
                        nc.sync.dma_start(out=b2g2_sb[:], in_=b2g2_d[:, :])
                        ahi0 = NA_PAD // ATOM_CHUNKS
                        nc.sync.dma_start(
                            out=atom_sb[:, 0:ahi0], in_=atomT_d[:, 0:ahi0]
                        )
                        nc.sync.dma_start(
                            out=xt_sb[:, 2048:4096], in_=xt_d[:, 2048:4096]
                        )
                        nc.sync.dma_start(
                            out=xt_sb[:, 4096:w], in_=xt_d[:, 4096:w]
                        )
                    else:
                        nc.sync.dma_start(
                            out=xt_sb[:, :w],
                            in_=xt_d[:, m2 * 512: m2 * 512 + w],
                        )
                    s8_sb = s8p.tile([TPB, CHUNK_B * 512], _BF16, tag="s8")
                    c0 = m2 * L2B * BLK
                    ws = min(CHUNK_B, nb2 - m2) * 512
                    nc.sync.dma_start(
                        out=s8_sb[:, :ws], in_=s8_d[:, c0:c0 + ws]
                    )
                    s8_chunk.insert(0, (s8_sb, c0))
                    del s8_chunk[2:]

                # ---- L1: two K=64 row-packed MMs (PE rows 0-63 / 64-127
                # stream concurrently), one FD=1024 silu over both halves
                co = (m2 % CHUNK_B) * 512
                psum1 = psum1p.tile([2 * D, 1024], _F32, tag="psum1")
                nc.tensor.matmul(
                    psum1[:, 0:512], wg1_sb[0:D, :], xt_sb[0:D, co:co + 512],
                    start=True, stop=True,
                )
                nc.tensor.matmul(
                    psum1[:, 512:1024], wg1_sb[D:2 * D, :],
                    xt_sb[D:2 * D, co:co + 512],
                    start=True, stop=True,
                )
                act1 = actp.tile([2 * D, 1024], _BF16, tag="act1")
                if has_bias1:
                    nc.scalar.activation(
                        act1[:], psum1[:], AF.Silu, bias=b1g1_sb[:, 0:1],
                        scale=1.0,
                    )
                else:
                    nc.scalar.activation(act1[:], psum1[:], AF.Silu)
                stash_act[m2] = act1

                # post-L2 stages for the pair completed two batches ago:
                # one h2 half after each act1 so both act1->act1
                # transitions on ACT are covered while PE runs L1
                if m2 >= 4 and m2 % 2 == 0:
                    emit_post_a(m2 // 2 - 2)
                if m2 >= 5 and m2 % 2 == 1:
                    emit_post_b((m2 - 1) // 2 - 2)

                # ---- L2 for batch m2-1 (act1 one iteration old, so the
                # act1->L2->h2 chain spans iterations instead of serializing
                # inside one period)
                if m2 >= 1:
                    emit_l2(m2 - 1)

                # ---- atom features arrive in chunks during the first
                # iterations instead of one serial 3.2MB DMA before compute
                if 1 <= m2 < ATOM_CHUNKS:
                    alo = (m2 * NA_PAD) // ATOM_CHUNKS
                    ahi = ((m2 + 1) * NA_PAD) // ATOM_CHUNKS
                    nc.sync.dma_start(
                        out=atom_sb[:, alo:ahi], in_=atomT_d[:, alo:ahi]
                    )

                # ---- segment accumulation for pair m2//2 - 3
                if m2 >= 6 and m2 % 2 == 0:
                    emit_seg(m2 // 2 - 3)

            emit_l2(nb2 - 1)
            nq = nb2 // 2
            for q in range(nq - 2, nq):
                emit_post_a(q)
                emit_post_b(q)
            for q in range(nq - 3, nq):
                emit_seg(q)

    # run_bass_kernel_spmd's serialization path skips the Bacc compile step
    # that fills in the 64-byte ISA payload of InstCustomDveAnt; without
    # this, walrus codegen fails with "ISA wrong length".
    mybir.codegen_inst_isa_subclasses(nc)
    return nc


# ----------------------------------------------------------------- kernel()
LAST_EXEC_NS = None
LAST_RESULT = None


def kernel(**inputs):
    atom_features = np.asarray(inputs["atom_features"], dtype=np.float32)
    bond_features = np.asarray(inputs["bond_features"], dtype=np.float32)
    bond_atom_indices = np.asarray(inputs["bond_atom_indices"])
    W1 = np.asarray(inputs["W1"], dtype=np.float32)
    W2 = np.asarray(inputs["W2"], dtype=np.float32)
    G1 = np.asarray(inputs["G1"], dtype=np.float32)
    G2 = np.asarray(inputs["G2"], dtype=np.float32)
    b1 = np.asarray(inputs["b1"], dtype=np.float32)
    b2 = np.asarray(inputs["b2"], dtype=np.float32)
    g1 = np.asarray(inputs["g1"], dtype=np.float32)
    g2 = np.asarray(inputs["g2"], dtype=np.float32)

    ntiles, tile_block, xt_list, s8_list, atomT_list = _plan_and_pack(
        atom_features, bond_features, bond_atom_indices
    )
    wg1, wg2, b1g1, b2g2 = _pack_weights(W1, G1, W2, G2, b1, g1, b2, g2)
    has_bias1 = not (np.all(b1 == 0.0) and np.all(g1 == 0.0))
    has_bias2 = not (np.all(b2 == 0.0) and np.all(g2 == 0.0))

    nc = _build_nc(ntiles, tile_block, has_bias1, has_bias2)

    in_maps = []
    for c in range(N_CORES):
        in_maps.append({
            "xt": xt_list[c],
            "s8": s8_list[c],
            "atomT": atomT_list[c],
            "wg1": wg1,
            "wg2": wg2,
            "b1g1": b1g1,
            "b2g2": b2g2,
        })

    import os as _os
    _trace = bool(int(_os.environ.get("KERNEL_TRACE", "0")))
    res = run_bass_kernel_spmd(nc, in_maps, core_ids=list(range(N_CORES)), trace=_trace)
    global LAST_EXEC_NS, LAST_RESULT
    LAST_EXEC_NS = res.exec_time_ns
    LAST_RESULT = res

    out = np.empty((N_ATOMS, D), dtype=np.float32)
    for c in range(N_CORES):
        out[c * NA_CORE:(c + 1) * NA_CORE] = res.results[c]["out"][:, :NA_CORE].T
    return out


# revision 24
# speedup vs baseline: 1.9263x; 1.8184x over previous
"""Trainium2 Bass kernel for nn_GatedAtomUpdate (gnn_message_passing).

Strategy (no collectives needed):
  - Host sorts bonds by receiver atom and buckets them into 8 contiguous
    atom ranges (12500 atoms/core). Each core computes the gated MLP for
    its own bonds and segment-sums locally into its own atom slice; the
    host concatenates the 8 output slices. No all-reduce.
  - Bonds are packed into 128-bond tiles, each tile assigned to a single
    64-atom block (pad bonds get an all-zero one-hot row).
  - The ACT (scalar) engine is the bottleneck: the math needs 4
    activation evals per bond-feature (silu h1, silu g1, silu h2,
    sigmoid g2) and ACT runs 1 elem/lane/cycle + ~222cyc/instr overhead.
    Three changes vs the 453us baseline:
      1. The gate sigmoid moves OFF ACT onto the DVE as a calibrated
         clamped affine: sigma(x) ~= clamp(a*x + 0.5, 0, 1). Two
         tensor_scalar ops (one reading PSUM directly).
      2. The one-hot scatter matrix S8 is precomputed on the HOST and
         DMA'd in (DMA has ~3x headroom), removing the per-tile
         IS_EQ build from DVE.
      3. L2 outputs for a PAIR of batches accumulate into one PSUM tile
         [128, 16, 128] so the h2-silu runs as one FD=1024 ACT
         instruction per pair, halving its per-instr overhead.
  - Device pipeline per 1024-bond batch:
      L1:  psum1[128(h|g), 1024] = [W1|G1]^T @ x^T   (two K=64 row-group
           MMs, row-packed at PE tile rows 0/64 -> stream concurrently)
      ACT: act1 = silu(psum1 + [b1;g1])              (one FD=1024 instr)
      L2:  psum2pair[128 bonds, 16, 128]: per-tile MM, stationary=act1
           tile, moving=blockdiag(W2,G2) -> bond-major [h2pre | g2pre]
      per PAIR of batches:
      ACT: h2 = silu(h2pre)                          (one FD=1024 instr)
      DVE: u    = a*g2pre + 0.5   (tensor_scalar, PSUM src)
           u    = clamp(u, 0, 1)  (tensor_scalar, 4x)
           msg  = h2 * u          (tensor_tensor, 2x)
      SEG: PE matmul msg^T @ S8 accumulates [64 feat, 64 atom] PSUM
           blocks; on block close DVE adds the atom_features slice.
"""

import math

import numpy as np
import ml_dtypes

import bass_rust
import concourse.bass as bass
import concourse.mybir as mybir
import concourse.tile as tile
from concourse.bass_utils import run_bass_kernel_spmd


def _ensure_axon_hooks():
    """bass_utils imports antenv.axon_hooks when tracing is requested (e.g.
    BASS_TRACE in the environment). Some images lack that module; install a
    graceful fallback so the kernel still runs (tracing is skipped when the
    injected libaxon has no profile symbols)."""
    try:
        import antenv.axon_hooks  # noqa: F401
        return
    except Exception:
        pass
    try:
        import sys
        import types

        import antenv
    except Exception:
        return
    mod = types.ModuleType("antenv.axon_hooks")
    _box = [None]
    mod.set_axon_ntff_profile_hook = lambda h: _box.__setitem__(0, h)
    mod.get_axon_ntff_profile_hook = lambda: _box[0]
    try:
        import contextlib
        import ctypes

        lib = ctypes.CDLL("/opt/axon/libaxon_pjrt.so")
        if hasattr(lib, "axon_start_nrt_profile"):
            lib.axon_start_nrt_profile.argtypes = [
                ctypes.POINTER(ctypes.c_int64),
                ctypes.c_size_t,
            ]
            lib.axon_start_nrt_profile.restype = ctypes.c_int64
            lib.axon_stop_nrt_profile.argtypes = [ctypes.c_char_p]
            lib.axon_stop_nrt_profile.restype = ctypes.c_int64

            @contextlib.contextmanager
            def _hook(output_dir, device_ids):
                import jax

                jax.devices()
                if device_ids:
                    ids = (ctypes.c_int64 * len(device_ids))(*device_ids)
                    rc = lib.axon_start_nrt_profile(ids, len(device_ids))
                else:
                    rc = lib.axon_start_nrt_profile(None, 0)
                if rc != 0:
                    raise RuntimeError(f"axon_start_nrt_profile rc={rc}")
                try:
                    yield
                finally:
                    lib.axon_stop_nrt_profile(str(output_dir).encode())

            _box[0] = _hook
    except Exception:
        pass
    sys.modules["antenv.axon_hooks"] = mod
    antenv.axon_hooks = mod


_ensure_axon_hooks()

# ---------------------------------------------------------------- constants
N_CORES = 8
D = 64
N_ATOMS = 100000
N_BONDS = 1500000
NA_CORE = N_ATOMS // N_CORES          # 12500
BLK = 64                              # atoms per block (one-hot width)
NBLK = math.ceil(NA_CORE / BLK)       # 196 blocks/core
NA_PAD = NBLK * BLK                   # 12544
TPB = 128                             # bonds per tile
L2B = 8                               # tiles per batch (1024 bonds)
CHUNK_B = 16                          # batches per xt/s8 DMA chunk
ATOM_CHUNKS = 8                       # atom-feature DMA split (ramp overlap)
OUT_CHUNKS = 16                       # output DMA split (tail overlap)

_BF16 = mybir.dt.bfloat16
_F32 = mybir.dt.float32


# --------------------------------------------- custom DVE op: 2*sigmoid(x)
# f2(x) = 1 + s*(C2 - |s|),  s = clamp(C0*x, -C1, C1)  ~=  2*sigmoid(x)
# (piecewise-quadratic, monotone, saturates at ~0/2 for all x; max err
# ~0.013 in sigma units). The doubled output is undone in the final
# atom add (affine_then_add with scale=0.5), so the gate multiply stays
# a stock 2x tensor_tensor. Constants minimize max |f2/2 - sigmoid|.
GATE_C0 = 0.26600
GATE_C1 = 0.92000
GATE_C2 = 1.97570


def _register_gate_op():
    import concourse.dve_ops as dve_ops
    from concourse.dve_spec import (
        Spec, Src0, C0, C1, C2, Zero, One, lower, maxx, minn,
        _has_src1,
    )
    from concourse.dve_uop import DveOpSpec

    name = "GATE2_SIG_ANT"
    for op in dve_ops.OPS:
        if op.name == name:
            return op

    t = Src0 * C0
    s = minn(maxx(t, Zero - C1), C1)
    a = maxx(s, Zero - s)
    body = One + s * (C2 - a)

    def _ref(in0, in1, s0, s1, imm2):
        ss = np.clip(np.asarray(in0, np.float32) * s0, -s1, s1)
        return (1.0 + ss * (imm2 - np.abs(ss))).astype(np.float32)

    spec = Spec(body=body, reference=_ref)
    row = dve_ops._CUSTOM_DVE_ROW_BASE + len(dve_ops.OPS)
    shas = {}
    for ver in ("v3", "v4"):
        tmp = DveOpSpec(
            name=name, opcode=row, uops=lower(spec, ver=ver),
            rd1_en=_has_src1(spec),
        )
        shas[ver] = tmp.sha(ver)
    gate = dve_ops.DveOp(name, spec, subdim=False, uops_sha=shas)
    dve_ops.OPS.append(gate)
    dve_ops._SUB_OPCODE_FOR_NAME[name] = row
    dve_ops.CUSTOM_DVE_SPECS[name] = spec
    return gate


GATE2_OP = _register_gate_op()


# ------------------------------------------------------- walrus workaround
def _patched_drain_and_barrier(self, tick_clock, wait_clock):
    """This walrus build accepts at most ONE sync-wait on TPB_CTRL-class
    instructions (Drain/NoOp), but TileContext's exit drain attaches one
    wait per DMA completion lane. Emit the waits on single-wait NOPs on
    the same engine first (program order gives the same guarantee), leave
    the drain bare, and reset semaphores one at a time."""
    nc = self.nc
    gc = tick_clock.global_clock
    ticks = list(gc)
    n = len(ticks)
    for proc, t in enumerate(ticks):
        if t > 0:
            vcp = bass_rust.VectorClock([t if j == proc else 0 for j in range(n)])
            nop = nc.sync.nop()
            wait_clock.add_sem_waits(nop.ins, tile.ScopedClock({None: vcp}))
    nc.sync.drain()
    nc.all_engine_barrier()
    assert self.sems is not None
    popped = nc._tile_sem_poison_stack.pop()
    assert popped is self._sem_poison
    for s in list(self.sems.allocated().values()):
        nc.clear_and_free_semaphores([s])
    nc.all_engine_barrier()


tile.TileContext._drain_and_barrier = _patched_drain_and_barrier


def _split_multi_waits(bir):
    """This walrus build rejects >1 sync-wait on an instruction ('Too many
    sync wait commands'). Move extra waits onto fresh single-wait NoOps
    inserted immediately before the instruction on the same engine —
    program order on the engine's sequencer preserves semantics."""
    n_new = 0
    for fn in bir.get("functions", []):
        for bb in fn.get("blocks", []):
            insts = bb.get("instructions", [])
            out = []
            for inst in insts:
                si = inst.get("sync_info") or {}
                ow = si.get("on_wait") or []
                if len(ow) > 1:
                    for i, w in enumerate(ow[:-1]):
                        out.append({
                            "name": f"{inst['name']}_sw{i}",
                            "opcode": "NoOp",
                            "engine": inst["engine"],
                            "ins": [],
                            "outs": [],
                            "sync_info": {"on_update": [], "on_wait": [w]},
                            "debug": inst.get("debug", 0),
                        })
                        n_new += 1
                    si["on_wait"] = [ow[-1]]
                out.append(inst)
            bb["instructions"] = out
    return n_new


_orig_to_json_bytes = bass.Bass.to_json_bytes


def _to_json_bytes_patched(self, *args, **kwargs):
    import json as _json
    raw = _orig_to_json_bytes(self, *args, **kwargs)
    bir = _json.loads(raw)
    n = _split_multi_waits(bir)
    if n == 0:
        return raw
    return _json.dumps(bir).encode()


bass.Bass.to_json_bytes = _to_json_bytes_patched


# ------------------------------------------------------------ host sharding
def _plan_and_pack(atom_features, bond_features, bond_atom_indices):
    """Sort bonds by receiver, bucket to cores/blocks, build a tile schedule
    shared by all cores (SPMD: one instruction stream), and pack per-core
    input arrays (bond features + host-built one-hot scatter tiles)."""
    recv = bond_atom_indices[:, 1].astype(np.int64)
    order = np.argsort(recv, kind="stable")
    sorted_recv = recv[order]
    core_edges = np.searchsorted(sorted_recv, np.arange(N_CORES + 1) * NA_CORE)

    # per-core, per-block bond counts
    cnt = np.zeros((N_CORES, NBLK), dtype=np.int64)
    locals_ = []
    for c in range(N_CORES):
        lo, hi = core_edges[c], core_edges[c + 1]
        local = sorted_recv[lo:hi] - c * NA_CORE
        locals_.append(local)
        cnt[c] = np.bincount(local // BLK, minlength=NBLK)

    # shared tile schedule: tiles per block (>=1 so every block is written);
    # total padded to an even number of batches (paired psum2 tiles)
    T = np.maximum(1, -(-cnt.max(axis=0) // TPB))
    ntiles = int(T.sum())
    pad_tiles = (-ntiles) % (2 * L2B)
    T[-1] += pad_tiles
    ntiles += pad_tiles
    tstart = np.concatenate([[0], np.cumsum(T)[:-1]]).astype(np.int64)

    # block id for every tile, in order
    tile_block = np.repeat(np.arange(NBLK), T)

    xt_list, s8_list = [], []
    nslots = ntiles * TPB
    arange64 = np.arange(BLK, dtype=np.float32)
    for c in range(N_CORES):
        local = locals_[c]
        blk = local // BLK
        block_off = np.concatenate([[0], np.cumsum(cnt[c])[:-1]])
        off_in_block = np.arange(local.shape[0]) - block_off[blk]
        slot = tstart[blk] * TPB + off_in_block

        gather = np.full(nslots, -1, dtype=np.int64)
        gather[slot] = order[core_edges[c]:core_edges[c + 1]]
        rel = np.full(nslots, 255.0, dtype=np.float32)
        rel[slot] = (local - blk * BLK).astype(np.float32)

        x_slot = np.zeros((nslots, D), dtype=np.float32)
        valid = gather >= 0
        x_slot[valid] = bond_features[gather[valid]]

        # pack into 128 partitions: row h*64+f, col m2*512+j holds
        # feature f of bond slot m2*1024 + h*512 + j  (h = 0/1 selects the
        # PE row-group the L1 matmul for that half streams from)
        nb2 = ntiles // L2B
        xs = x_slot.reshape(nb2, 2, 512, D)
        xt = np.ascontiguousarray(
            xs.transpose(1, 3, 0, 2).reshape(2 * D, nb2 * 512)
        ).astype(ml_dtypes.bfloat16)
        xt_list.append(xt)

        # host-built one-hot: s8[p, t*64 + a] = (rel[t*128+p] == a)
        rel2 = rel.reshape(ntiles, TPB).T                     # [128, ntiles]
        s8 = (rel2[:, :, None] == arange64[None, None, :])
        s8 = np.ascontiguousarray(
            s8.reshape(TPB, ntiles * BLK)
        ).astype(ml_dtypes.bfloat16)
        s8_list.append(s8)

    atomT_list = []
    for c in range(N_CORES):
        ap = np.zeros((NA_PAD, D), dtype=np.float32)
        ap[:NA_CORE] = atom_features[c * NA_CORE:(c + 1) * NA_CORE]
        atomT_list.append(np.ascontiguousarray(ap.T))

    return ntiles, tile_block, xt_list, s8_list, atomT_list


def _pack_weights(W1, G1, W2, G2, b1, g1, b2, g2):
    wg1_row = np.concatenate([W1, G1], axis=1)              # [64, 128]
    wg1 = np.concatenate([wg1_row, wg1_row], axis=0)        # [128, 128]
    wg2 = np.zeros((2 * D, 2 * D), dtype=np.float32)
    wg2[:D, :D] = W2
    wg2[D:, D:] = G2
    b1g1 = np.concatenate([b1, g1]).reshape(2 * D, 1).astype(np.float32)
    b2g2 = np.concatenate([b2, g2]).reshape(1, 2 * D)
    return (
        wg1.astype(ml_dtypes.bfloat16),
        wg2.astype(ml_dtypes.bfloat16),
        b1g1,
        b2g2.astype(ml_dtypes.bfloat16),
    )


# ------------------------------------------------------------- device kernel
def _build_nc(ntiles, tile_block, has_bias1, has_bias2):
    nb2 = ntiles // L2B
    assert nb2 % 2 == 0
    nc = bass.Bass()

    xt_d = nc.dram_tensor("xt", [2 * D, nb2 * 512], _BF16, kind="ExternalInput")
    s8_d = nc.dram_tensor("s8", [TPB, ntiles * BLK], _BF16, kind="ExternalInput")
    atomT_d = nc.dram_tensor("atomT", [D, NA_PAD], _F32, kind="ExternalInput")
    wg1_d = nc.dram_tensor("wg1", [2 * D, 2 * D], _BF16, kind="ExternalInput")
    wg2_d = nc.dram_tensor("wg2", [2 * D, 2 * D], _BF16, kind="ExternalInput")
    b1g1_d = nc.dram_tensor("b1g1", [2 * D, 1], _F32, kind="ExternalInput")
    b2g2_d = nc.dram_tensor("b2g2", [1, 2 * D], _BF16, kind="ExternalInput")
    out_d = nc.dram_tensor("out", [D, NA_PAD], _F32, kind="ExternalOutput")

    AF = mybir.ActivationFunctionType

    # first/last tile flags per block
    first_of_block = np.zeros(ntiles, dtype=bool)
    last_of_block = np.zeros(ntiles, dtype=bool)
    prev = -1
    for t in range(ntiles):
        b = tile_block[t]
        if b != prev:
            first_of_block[t] = True
            if t > 0:
                last_of_block[t - 1] = True
            prev = b
    last_of_block[ntiles - 1] = True

    with tile.TileContext(nc) as tc:
        with (
            tc.tile_pool(name="singles", bufs=1) as singles,
            tc.tile_pool(name="xtp", bufs=2) as xtp,
            tc.tile_pool(name="s8p", bufs=2) as s8p,
            tc.tile_pool(name="actp", bufs=3) as actp,
            tc.tile_pool(name="l2p", bufs=2) as l2p,
            # PSUM budget (8 banks): psum1 [128,1024]f32 x1 = 2, psum2
            # [128,8,128]f32 x2 bufs = 4, pseg x2 = 2 — the baseline's
            # proven layout. psum2 double-buffering gives the post-L2
            # stage a 2-batch window, and the per-batch h2 silu between
            # consecutive act1's hides the L1 matmul latency on ACT.
            tc.tile_pool(name="psum1p", bufs=1, space="PSUM") as psum1p,
            tc.tile_pool(name="psum2p", bufs=2, space="PSUM") as psum2p,
            tc.tile_pool(name="psegp", bufs=2, space="PSUM") as psegp,
        ):
            # Each dma_start doorbell costs ~650ns of serial Sync-queue issue
            # time; the first L1 matmul needs only wg1 + the first xt
            # mini-chunk, so only wg1's DMA is emitted here and everything
            # else is deferred into the m2==0 branch after the xt mini-chunk.
            wg1_sb = singles.tile([2 * D, 2 * D], _BF16)
            nc.sync.dma_start(out=wg1_sb[:], in_=wg1_d[:, :])
            wg2_sb = singles.tile([2 * D, 2 * D], _BF16)
            b1g1_sb = singles.tile([2 * D, 1], _F32)
            b2g2_sb = singles.tile([1, 2 * D], _BF16)
            ones_sb = singles.tile([1, 2 * D], _BF16)
            nc.vector.memset(ones_sb[:], 1.0)
            atom_sb = singles.tile([D, NA_PAD], _F32)
            out_sb = singles.tile([D, NA_PAD], _F32)

            pseg_cur = None
            stash_act = {}    # m -> act1 handle awaiting L2
            stash_post = {}   # m -> psum2 tile awaiting act+mult
            stash = {}        # m -> msg handle awaiting segment acc

            def emit_l2(j):
                """L2 matmuls for batch j: per-tile stationary=act1 slice,
                moving=blockdiag(W2|G2)."""
                a = stash_act.pop(j)
                psum2 = psum2p.tile([TPB, L2B, 2 * D], _F32, tag="psum2")
                for tt in range(L2B):
                    sl = tt * TPB
                    nc.tensor.matmul(
                        psum2[:, tt, :], a[:, sl:sl + TPB], wg2_sb[:, :],
                        start=True, stop=not has_bias2,
                    )
                    if has_bias2:
                        nc.tensor.matmul(
                            psum2[:, tt, :], ones_sb[0:1, :],
                            b2g2_sb[0:1, :],
                            start=False, stop=True,
                        )
                stash_post[j] = psum2

            def emit_post(j):
                """Post-L2 stage: silu on ACT for h2pre, one fused
                custom-DVE op for ~2*sigmoid(g2pre), then the gate
                multiply (msg = 2x true message; the final atom add
                rescales by 0.5). Emitted one batch late so inputs are
                ready and the ACT queue never stalls."""
                psum2_j = stash_post.pop(j)
                h2 = l2p.tile([TPB, L2B, D], _BF16, tag="h2")
                nc.scalar.activation(h2[:], psum2_j[:, :, 0:D], AF.Silu)
                f2 = l2p.tile([TPB, L2B, D], _BF16, tag="f2")
                nc.vector._custom_dve(
                    GATE2_OP, out=f2[:], in0=psum2_j[:, :, D:2 * D],
                    s0=GATE_C0, s1=GATE_C1, imm2=GATE_C2,
                )
                msg = l2p.tile([TPB, L2B, D], _BF16, tag="msg")
                nc.vector.tensor_tensor(msg[:], h2[:], f2[:], mybir.AluOpType.mult)
                stash[j] = msg

            # blocks close in tile order; stream the output back to HBM in
            # chunks as soon as the last block of each chunk is done so the
            # final transfer isn't serialized after the last matmul
            out_edges = [
                (k * NBLK) // OUT_CHUNKS for k in range(1, OUT_CHUNKS + 1)
            ]

            def emit_seg(j):
                nonlocal pseg_cur
                msg_q = stash.pop(j)
                t0q = j * L2B
                # a batch's tiles live in one 16-batch s8 chunk; pick the
                # kept handle that covers this batch's columns
                want = t0q * BLK
                s8_sb, s8_c0 = next(
                    (sb, c0) for (sb, c0) in s8_chunk
                    if c0 <= want < c0 + CHUNK_B * 512
                )
                for tt in range(L2B):
                    t_glob = t0q + tt
                    b = int(tile_block[t_glob])
                    if first_of_block[t_glob]:
                        pseg_cur = psegp.tile([D, BLK], _F32, tag="pseg")
                    so = t_glob * BLK - s8_c0
                    nc.tensor.matmul(
                        pseg_cur[:, :], msg_q[:, tt, :],
                        s8_sb[:, so:so + BLK],
                        start=bool(first_of_block[t_glob]),
                        stop=bool(last_of_block[t_glob]),
                    )
                    if last_of_block[t_glob]:
                        # messages were doubled by the gate op; rescale here
                        nc.vector.affine_then_add(
                            out_sb[:, b * BLK:(b + 1) * BLK],
                            pseg_cur[:, :],
                            atom_sb[:, b * BLK:(b + 1) * BLK],
                            scale=0.5, bias=0.0,
                        )
                        if b + 1 in out_edges:
                            lo = out_edges[out_edges.index(b + 1) - 1] * BLK \
                                if out_edges.index(b + 1) > 0 else 0
                            nc.sync.dma_start(
                                out=out_d[:, lo:(b + 1) * BLK],
                                in_=out_sb[:, lo:(b + 1) * BLK],
                            )

            # s8 chunk bookkeeping: emit_seg(q) runs ~5 batches behind the
            # xt chunk loop, so the s8 chunk DMA is issued on the same
            # 16-batch cadence but the chunk consumed by emit_seg is the
            # PREVIOUS one for the first 5 batches of each chunk window.
            # Keep the last two chunk handles.
            s8_chunk = []  # [(tile, col0)], newest first

            for m2 in range(nb2):
                # ---- xt + s8 chunk DMA (every CHUNK_B batches); the first
                # chunk is split so compute starts after 512KB instead of
                # 2MB; s8 is only needed by emit_seg (runs ~5 batches late)
                if m2 % CHUNK_B == 0:
                    w = min(CHUNK_B, nb2 - m2) * 512
                    xt_sb = xtp.tile([2 * D, CHUNK_B * 512], _BF16, tag="xt")
                    if m2 == 0:
                        nc.sync.dma_start(
                            out=xt_sb[:, 0:2048], in_=xt_d[:, 0:2048]
                        )
                        nc.sync.dma_start(out=b1g1_sb[:], in_=b1g1_d[:, :])
                        nc.sync.dma_start(out=wg2_sb[:], in_=wg2_d[:, :])
                        nc.sync.dma_start(out=b2g2_sb[:], in_=b2g2_d[:, :])
                        ahi0 = NA_PAD // ATOM_CHUNKS
                        nc.sync.dma_start(
                            out=atom_sb[:, 0:ahi0], in_=atomT_d[:, 0:ahi0]
                        )
                        nc.sync.dma_start(
                            out=xt_sb[:, 2048:4096], in_=xt_d[:, 2048:4096]
                        )
                        nc.sync.dma_start(
                            out=xt_sb[:, 4096:w], in_=xt_d[:, 4096:w]
                        )
                    else:
                        nc.sync.dma_start(
                            out=xt_sb[:, :w],
                            in_=xt_d[:, m2 * 512: m2 * 512 + w],
                        )
                    s8_sb = s8p.tile([TPB, CHUNK_B * 512], _BF16, tag="s8")
                    c0 = m2 * L2B * BLK
                    ws = min(CHUNK_B, nb2 - m2) * 512
                    nc.sync.dma_start(
                        out=s8_sb[:, :ws], in_=s8_d[:, c0:c0 + ws]
                    )
                    s8_chunk.insert(0, (s8_sb, c0))
                    del s8_chunk[2:]

                # ---- L1: two K=64 row-packed MMs (PE rows 0-63 / 64-127
                # stream concurrently), one FD=1024 silu over both halves
                co = (m2 % CHUNK_B) * 512
                psum1 = psum1p.tile([2 * D, 1024], _F32, tag="psum1")
                nc.tensor.matmul(
                    psum1[:, 0:512], wg1_sb[0:D, :], xt_sb[0:D, co:co + 512],
                    start=True, stop=True,
                )
                nc.tensor.matmul(
                    psum1[:, 512:1024], wg1_sb[D:2 * D, :],
                    xt_sb[D:2 * D, co:co + 512],
                    start=True, stop=True,
                )
                act1 = actp.tile([2 * D, 1024], _BF16, tag="act1")
                if has_bias1:
                    nc.scalar.activation(
                        act1[:], psum1[:], AF.Silu, bias=b1g1_sb[:, 0:1],
                        scale=1.0,
                    )
                else:
                    nc.scalar.activation(act1[:], psum1[:], AF.Silu)
                stash_act[m2] = act1

                # post-L2 stage for batch m2-2, right after act1 in the
                # ACT queue (inputs two iterations old - ACT never stalls)
                if m2 >= 2:
                    emit_post(m2 - 2)

                # ---- L2 for batch m2-1 (act1 one iteration old, so the
                # act1->L2->h2 chain spans iterations instead of serializing
                # inside one period)
                if m2 >= 1:
                    emit_l2(m2 - 1)

                # ---- atom features arrive in chunks during the first
                # iterations instead of one serial 3.2MB DMA before compute
                if 1 <= m2 < ATOM_CHUNKS:
                    alo = (m2 * NA_PAD) // ATOM_CHUNKS
                    ahi = ((m2 + 1) * NA_PAD) // ATOM_CHUNKS
                    nc.sync.dma_start(
                        out=atom_sb[:, alo:ahi], in_=atomT_d[:, alo:ahi]
                    )

                # ---- segment accumulation for batch m2-3
                if m2 >= 3:
                    emit_seg(m2 - 3)

            emit_l2(nb2 - 1)
            emit_post(nb2 - 2)
            if nb2 >= 3:
                emit_seg(nb2 - 3)
            emit_post(nb2 - 1)
            emit_seg(nb2 - 2)
            emit_seg(nb2 - 1)

    # run_bass_kernel_spmd's serialization path skips the Bacc compile step
    # that fills in the 64-byte ISA payload of InstCustomDveAnt; without
    # this, walrus codegen fails with "ISA wrong length".
    mybir.codegen_inst_isa_subclasses(nc)
    return nc


# ----------------------------------------------------------------- kernel()
LAST_EXEC_NS = None
LAST_RESULT = None


def kernel(**inputs):
    atom_features = np.asarray(inputs["atom_features"], dtype=np.float32)
    bond_features = np.asarray(inputs["bond_features"], dtype=np.float32)
    bond_atom_indices = np.asarray(inputs["bond_atom_indices"])
    W1 = np.asarray(inputs["W1"], dtype=np.float32)
    W2 = np.asarray(inputs["W2"], dtype=np.float32)
    G1 = np.asarray(inputs["G1"], dtype=np.float32)
    G2 = np.asarray(inputs["G2"], dtype=np.float32)
    b1 = np.asarray(inputs["b1"], dtype=np.float32)
    b2 = np.asarray(inputs["b2"], dtype=np.float32)
    g1 = np.asarray(inputs["g1"], dtype=np.float32)
    g2 = np.asarray(inputs["g2"], dtype=np.float32)

    ntiles, tile_block, xt_list, s8_list, atomT_list = _plan_and_pack(
        atom_features, bond_features, bond_atom_indices
    )
    wg1, wg2, b1g1, b2g2 = _pack_weights(W1, G1, W2, G2, b1, g1, b2, g2)
    has_bias1 = not (np.all(b1 == 0.0) and np.all(g1 == 0.0))
    has_bias2 = not (np.all(b2 == 0.0) and np.all(g2 == 0.0))

    nc = _build_nc(ntiles, tile_block, has_bias1, has_bias2)

    in_maps = []
    for c in range(N_CORES):
        in_maps.append({
            "xt": xt_list[c],
            "s8": s8_list[c],
            "atomT": atomT_list[c],
            "wg1": wg1,
            "wg2": wg2,
            "b1g1": b1g1,
            "b2g2": b2g2,
        })

    import os as _os
    _trace = bool(int(_os.environ.get("KERNEL_TRACE", "0")))
    res = run_bass_kernel_spmd(nc, in_maps, core_ids=list(range(N_CORES)), trace=_trace)
    global LAST_EXEC_NS, LAST_RESULT
    LAST_EXEC_NS = res.exec_time_ns
    LAST_RESULT = res

    out = np.empty((N_ATOMS, D), dtype=np.float32)
    for c in range(N_CORES):
        out[c * NA_CORE:(c + 1) * NA_CORE] = res.results[c]["out"][:, :NA_CORE].T
    return out


# revision 30
# speedup vs baseline: 1.9747x; 1.0251x over previous
"""Trainium2 Bass kernel for nn_GatedAtomUpdate (gnn_message_passing).

Strategy (no collectives needed):
  - Host sorts bonds by receiver atom and buckets them into 8 contiguous
    atom ranges (12500 atoms/core). Each core computes the gated MLP for
    its own bonds and segment-sums locally into its own atom slice; the
    host concatenates the 8 output slices. No all-reduce.
  - Bonds are packed into 128-bond tiles, each tile assigned to a single
    64-atom block (pad bonds get an all-zero one-hot row).
  - The ACT (scalar) engine is the bottleneck: the math needs 4
    activation evals per bond-feature (silu h1, silu g1, silu h2,
    sigmoid g2) and ACT runs 1 elem/lane/cycle + ~222cyc/instr overhead.
    Three changes vs the 453us baseline:
      1. The gate sigmoid moves OFF ACT onto the DVE as a calibrated
         clamped affine: sigma(x) ~= clamp(a*x + 0.5, 0, 1). Two
         tensor_scalar ops (one reading PSUM directly).
      2. The one-hot scatter matrix S8 is precomputed on the HOST and
         DMA'd in (DMA has ~3x headroom), removing the per-tile
         IS_EQ build from DVE.
      3. L2 outputs for a PAIR of batches accumulate into one PSUM tile
         [128, 16, 128] so the h2-silu runs as one FD=1024 ACT
         instruction per pair, halving its per-instr overhead.
  - Device pipeline per 1024-bond batch:
      L1:  psum1[128(h|g), 1024] = [W1|G1]^T @ x^T   (two K=64 row-group
           MMs, row-packed at PE tile rows 0/64 -> stream concurrently)
      ACT: act1 = silu(psum1 + [b1;g1])              (one FD=1024 instr)
      L2:  psum2pair[128 bonds, 16, 128]: per-tile MM, stationary=act1
           tile, moving=blockdiag(W2,G2) -> bond-major [h2pre | g2pre]
      per PAIR of batches:
      ACT: h2 = silu(h2pre)                          (one FD=1024 instr)
      DVE: u    = a*g2pre + 0.5   (tensor_scalar, PSUM src)
           u    = clamp(u, 0, 1)  (tensor_scalar, 4x)
           msg  = h2 * u          (tensor_tensor, 2x)
      SEG: PE matmul msg^T @ S8 accumulates [64 feat, 64 atom] PSUM
           blocks; on block close DVE adds the atom_features slice.
"""

import math

import numpy as np
import ml_dtypes

import bass_rust
import concourse.bass as bass
import concourse.mybir as mybir
import concourse.tile as tile
from concourse.bass_utils import run_bass_kernel_spmd


def _ensure_axon_hooks():
    """bass_utils imports antenv.axon_hooks when tracing is requested (e.g.
    BASS_TRACE in the environment). Some images lack that module; install a
    graceful fallback so the kernel still runs (tracing is skipped when the
    injected libaxon has no profile symbols)."""
    try:
        import antenv.axon_hooks  # noqa: F401
        return
    except Exception:
        pass
    try:
        import sys
        import types

        import antenv
    except Exception:
        return
    mod = types.ModuleType("antenv.axon_hooks")
    _box = [None]
    mod.set_axon_ntff_profile_hook = lambda h: _box.__setitem__(0, h)
    mod.get_axon_ntff_profile_hook = lambda: _box[0]
    try:
        import contextlib
        import ctypes

        lib = ctypes.CDLL("/opt/axon/libaxon_pjrt.so")
        if hasattr(lib, "axon_start_nrt_profile"):
            lib.axon_start_nrt_profile.argtypes = [
                ctypes.POINTER(ctypes.c_int64),
                ctypes.c_size_t,
            ]
            lib.axon_start_nrt_profile.restype = ctypes.c_int64
            lib.axon_stop_nrt_profile.argtypes = [ctypes.c_char_p]
            lib.axon_stop_nrt_profile.restype = ctypes.c_int64

            @contextlib.contextmanager
            def _hook(output_dir, device_ids):
                import jax

                jax.devices()
                if device_ids:
                    ids = (ctypes.c_int64 * len(device_ids))(*device_ids)
                    rc = lib.axon_start_nrt_profile(ids, len(device_ids))
                else:
                    rc = lib.axon_start_nrt_profile(None, 0)
                if rc != 0:
                    raise RuntimeError(f"axon_start_nrt_profile rc={rc}")
                try:
                    yield
                finally:
                    lib.axon_stop_nrt_profile(str(output_dir).encode())

            _box[0] = _hook
    except Exception:
        pass
    sys.modules["antenv.axon_hooks"] = mod
    antenv.axon_hooks = mod


_ensure_axon_hooks()

# ---------------------------------------------------------------- constants
N_CORES = 8
D = 64
N_ATOMS = 100000
N_BONDS = 1500000
NA_CORE = N_ATOMS // N_CORES          # 12500
BLK = 64                              # atoms per block (one-hot width)
NBLK = math.ceil(NA_CORE / BLK)       # 196 blocks/core
NA_PAD = NBLK * BLK                   # 12544
TPB = 128                             # bonds per tile
L2B = 8                               # tiles per batch (1024 bonds)
CHUNK_B = 16                          # batches per xt/s8 DMA chunk
ATOM_CHUNKS = 24                      # atom-feature DMA split (ramp overlap)
OUT_CHUNKS = 24                       # output DMA split (tail overlap)
N_WARMUP_MM = 20                      # dummy PE matmuls to raise the HAM
                                      # p-state before the first real L1

_BF16 = mybir.dt.bfloat16
_F32 = mybir.dt.float32


# --------------------------------------------- custom DVE op: 2*sigmoid(x)
# f2(x) = 1 + s*(C2 - |s|),  s = clamp(C0*x, -C1, C1)  ~=  2*sigmoid(x)
# (piecewise-quadratic, monotone, saturates at ~0/2 for all x; max err
# ~0.013 in sigma units). The doubled output is undone in the final
# atom add (affine_then_add with scale=0.5), so the gate multiply stays
# a stock 2x tensor_tensor. Constants minimize max |f2/2 - sigmoid|.
GATE_C0 = 0.26600
GATE_C1 = 0.92000
GATE_C2 = 1.97570


def _register_gate_op():
    import concourse.dve_ops as dve_ops
    from concourse.dve_spec import (
        Spec, Src0, C0, C1, C2, Zero, One, lower, maxx, minn,
        _has_src1,
    )
    from concourse.dve_uop import DveOpSpec

    name = "GATE2_SIG_ANT"
    for op in dve_ops.OPS:
        if op.name == name:
            return op

    t = Src0 * C0
    s = minn(maxx(t, Zero - C1), C1)
    a = maxx(s, Zero - s)
    body = One + s * (C2 - a)

    def _ref(in0, in1, s0, s1, imm2):
        ss = np.clip(np.asarray(in0, np.float32) * s0, -s1, s1)
        return (1.0 + ss * (imm2 - np.abs(ss))).astype(np.float32)

    spec = Spec(body=body, reference=_ref)
    row = dve_ops._CUSTOM_DVE_ROW_BASE + len(dve_ops.OPS)
    shas = {}
    for ver in ("v3", "v4"):
        tmp = DveOpSpec(
            name=name, opcode=row, uops=lower(spec, ver=ver),
            rd1_en=_has_src1(spec),
        )
        shas[ver] = tmp.sha(ver)
    gate = dve_ops.DveOp(name, spec, subdim=False, uops_sha=shas)
    dve_ops.OPS.append(gate)
    dve_ops._SUB_OPCODE_FOR_NAME[name] = row
    dve_ops.CUSTOM_DVE_SPECS[name] = spec
    return gate


GATE2_OP = _register_gate_op()


# ------------------------------------------------------- walrus workaround
def _patched_drain_and_barrier(self, tick_clock, wait_clock):
    """This walrus build accepts at most ONE sync-wait on TPB_CTRL-class
    instructions (Drain/NoOp), but TileContext's exit drain attaches one
    wait per DMA completion lane. Emit the waits on single-wait NOPs on
    the same engine first (program order gives the same guarantee), leave
    the drain bare, and reset semaphores one at a time."""
    nc = self.nc
    gc = tick_clock.global_clock
    ticks = list(gc)
    n = len(ticks)
    for proc, t in enumerate(ticks):
        if t > 0:
            vcp = bass_rust.VectorClock([t if j == proc else 0 for j in range(n)])
            nop = nc.sync.nop()
            wait_clock.add_sem_waits(nop.ins, tile.ScopedClock({None: vcp}))
    nc.sync.drain()
    nc.all_engine_barrier()
    assert self.sems is not None
    popped = nc._tile_sem_poison_stack.pop()
    assert popped is self._sem_poison
    for s in list(self.sems.allocated().values()):
        nc.clear_and_free_semaphores([s])
    nc.all_engine_barrier()


tile.TileContext._drain_and_barrier = _patched_drain_and_barrier


def _split_multi_waits(bir):
    """This walrus build rejects >1 sync-wait on an instruction ('Too many
    sync wait commands'). Move extra waits onto fresh single-wait NoOps
    inserted immediately before the instruction on the same engine —
    program order on the engine's sequencer preserves semantics."""
    n_new = 0
    for fn in bir.get("functions", []):
        for bb in fn.get("blocks", []):
            insts = bb.get("instructions", [])
            out = []
            for inst in insts:
                si = inst.get("sync_info") or {}
                ow = si.get("on_wait") or []
                if len(ow) > 1:
                    for i, w in enumerate(ow[:-1]):
                        out.append({
                            "name": f"{inst['name']}_sw{i}",
                            "opcode": "NoOp",
                            "engine": inst["engine"],
                            "ins": [],
                            "outs": [],
                            "sync_info": {"on_update": [], "on_wait": [w]},
                            "debug": inst.get("debug", 0),
                        })
                        n_new += 1
                    si["on_wait"] = [ow[-1]]
                out.append(inst)
            bb["instructions"] = out
    return n_new


_orig_to_json_bytes = bass.Bass.to_json_bytes


def _to_json_bytes_patched(self, *args, **kwargs):
    import json as _json
    raw = _orig_to_json_bytes(self, *args, **kwargs)
    bir = _json.loads(raw)
    n = _split_multi_waits(bir)
    if n == 0:
        return raw
    return _json.dumps(bir).encode()


bass.Bass.to_json_bytes = _to_json_bytes_patched


# ------------------------------------------------------------ host sharding
def _plan_and_pack(atom_features, bond_features, bond_atom_indices):
    """Sort bonds by receiver, bucket to cores/blocks, build a tile schedule
    shared by all cores (SPMD: one instruction stream), and pack per-core
    input arrays (bond features + host-built one-hot scatter tiles)."""
    recv = bond_atom_indices[:, 1].astype(np.int64)
    order = np.argsort(recv, kind="stable")
    sorted_recv = recv[order]
    core_edges = np.searchsorted(sorted_recv, np.arange(N_CORES + 1) * NA_CORE)

    # per-core, per-block bond counts
    cnt = np.zeros((N_CORES, NBLK), dtype=np.int64)
    locals_ = []
    for c in range(N_CORES):
        lo, hi = core_edges[c], core_edges[c + 1]
        local = sorted_recv[lo:hi] - c * NA_CORE
        locals_.append(local)
        cnt[c] = np.bincount(local // BLK, minlength=NBLK)

    # shared tile schedule: tiles per block (>=1 so every block is written);
    # total padded to a whole number of batches
    T = np.maximum(1, -(-cnt.max(axis=0) // TPB))
    ntiles = int(T.sum())
    pad_tiles = (-ntiles) % L2B
    T[-1] += pad_tiles
    ntiles += pad_tiles
    tstart = np.concatenate([[0], np.cumsum(T)[:-1]]).astype(np.int64)

    # block id for every tile, in order
    tile_block = np.repeat(np.arange(NBLK), T)

    xt_list, s8_list = [], []
    nslots = ntiles * TPB
    arange64 = np.arange(BLK, dtype=np.float32)
    for c in range(N_CORES):
        local = locals_[c]
        blk = local // BLK
        block_off = np.concatenate([[0], np.cumsum(cnt[c])[:-1]])
        off_in_block = np.arange(local.shape[0]) - block_off[blk]
        slot = tstart[blk] * TPB + off_in_block

        gather = np.full(nslots, -1, dtype=np.int64)
        gather[slot] = order[core_edges[c]:core_edges[c + 1]]
        rel = np.full(nslots, 255.0, dtype=np.float32)
        rel[slot] = (local - blk * BLK).astype(np.float32)

        x_slot = np.zeros((nslots, D), dtype=np.float32)
        valid = gather >= 0
        x_slot[valid] = bond_features[gather[valid]]

        # pack into 128 partitions: row h*64+f, col m2*512+j holds
        # feature f of bond slot m2*1024 + h*512 + j  (h = 0/1 selects the
        # PE row-group the L1 matmul for that half streams from)
        nb2 = ntiles // L2B
        xs = x_slot.reshape(nb2, 2, 512, D)
        xt = np.ascontiguousarray(
            xs.transpose(1, 3, 0, 2).reshape(2 * D, nb2 * 512)
        ).astype(ml_dtypes.bfloat16)
        xt_list.append(xt)

        # host-built one-hot: s8[p, t*64 + a] = (rel[t*128+p] == a)
        rel2 = rel.reshape(ntiles, TPB).T                     # [128, ntiles]
        s8 = (rel2[:, :, None] == arange64[None, None, :])
        s8 = np.ascontiguousarray(
            s8.reshape(TPB, ntiles * BLK)
        ).astype(ml_dtypes.bfloat16)
        s8_list.append(s8)

    atomT_list = []
    for c in range(N_CORES):
        ap = np.zeros((NA_PAD, D), dtype=np.float32)
        ap[:NA_CORE] = atom_features[c * NA_CORE:(c + 1) * NA_CORE]
        atomT_list.append(np.ascontiguousarray(ap.T))

    return ntiles, tile_block, xt_list, s8_list, atomT_list


def _pack_weights(W1, G1, W2, G2, b1, g1, b2, g2):
    wg1_row = np.concatenate([W1, G1], axis=1)              # [64, 128]
    wg1 = np.concatenate([wg1_row, wg1_row], axis=0)        # [128, 128]
    wg2 = np.zeros((2 * D, 2 * D), dtype=np.float32)
    wg2[:D, :D] = W2
    wg2[D:, D:] = G2
    b1g1 = np.concatenate([b1, g1]).reshape(2 * D, 1).astype(np.float32)
    b2g2 = np.concatenate([b2, g2]).reshape(1, 2 * D)
    return (
        wg1.astype(ml_dtypes.bfloat16),
        wg2.astype(ml_dtypes.bfloat16),
        b1g1,
        b2g2.astype(ml_dtypes.bfloat16),
    )


# ------------------------------------------------------------- device kernel
def _build_nc(ntiles, tile_block, has_bias1, has_bias2):
    nb2 = ntiles // L2B
    nc = bass.Bass()

    xt_d = nc.dram_tensor("xt", [2 * D, nb2 * 512], _BF16, kind="ExternalInput")
    s8_d = nc.dram_tensor("s8", [TPB, ntiles * BLK], _BF16, kind="ExternalInput")
    atomT_d = nc.dram_tensor("atomT", [D, NA_PAD], _F32, kind="ExternalInput")
    wg1_d = nc.dram_tensor("wg1", [2 * D, 2 * D], _BF16, kind="ExternalInput")
    wg2_d = nc.dram_tensor("wg2", [2 * D, 2 * D], _BF16, kind="ExternalInput")
    b1g1_d = nc.dram_tensor("b1g1", [2 * D, 1], _F32, kind="ExternalInput")
    b2g2_d = nc.dram_tensor("b2g2", [1, 2 * D], _BF16, kind="ExternalInput")
    out_d = nc.dram_tensor("out", [D, NA_PAD], _F32, kind="ExternalOutput")

    AF = mybir.ActivationFunctionType

    # first/last tile flags per block
    first_of_block = np.zeros(ntiles, dtype=bool)
    last_of_block = np.zeros(ntiles, dtype=bool)
    prev = -1
    for t in range(ntiles):
        b = tile_block[t]
        if b != prev:
            first_of_block[t] = True
            if t > 0:
                last_of_block[t - 1] = True
            prev = b
    last_of_block[ntiles - 1] = True

    with tile.TileContext(nc) as tc:
        with (
            tc.tile_pool(name="singles", bufs=1) as singles,
            tc.tile_pool(name="xtp", bufs=2) as xtp,
            tc.tile_pool(name="s8p", bufs=2) as s8p,
            tc.tile_pool(name="actp", bufs=3) as actp,
            tc.tile_pool(name="l2p", bufs=2) as l2p,
            # PSUM budget (8 banks): psum1 [128,1024]f32 x1 = 2, psum2
            # [128,8,128]f32 x2 bufs = 4, pseg x2 = 2 — the baseline's
            # proven layout. psum2 double-buffering gives the post-L2
            # stage a 2-batch window, and the per-batch h2 silu between
            # consecutive act1's hides the L1 matmul latency on ACT.
            tc.tile_pool(name="psum1p", bufs=1, space="PSUM") as psum1p,
            tc.tile_pool(name="psum2p", bufs=2, space="PSUM") as psum2p,
            tc.tile_pool(name="psegp", bufs=2, space="PSUM") as psegp,
        ):
            # Each dma_start doorbell costs ~650ns of serial Sync-queue issue
            # time; the first L1 matmul needs only wg1 + the first xt
            # mini-chunk, so only wg1's DMA is emitted here and everything
            # else is deferred into the m2==0 branch after the xt mini-chunk.
            wg1_sb = singles.tile([2 * D, 2 * D], _BF16)
            nc.sync.dma_start(out=wg1_sb[:], in_=wg1_d[:, :])
            wg2_sb = singles.tile([2 * D, 2 * D], _BF16)
            b1g1_sb = singles.tile([2 * D, 1], _F32)
            b2g2_sb = singles.tile([1, 2 * D], _BF16)
            ones_sb = singles.tile([1, 2 * D], _BF16)
            nc.vector.memset(ones_sb[:], 1.0)
            atom_sb = singles.tile([D, NA_PAD], _F32)
            out_sb = singles.tile([D, NA_PAD], _F32)

            # PE p-state warmup: the HAM clock-gates an idle PE down to
            # 0.65-1.2GHz and takes ~3us of sustained activity to release
            # full speed. Run dummy matmuls on a memset scratch (no DMA
            # dependency) during the initial input DMAs so the first real
            # batches execute at full clock. psum1's first tile is used as
            # a scratch target; the first real L1 starts with start=True
            # so the garbage never matters.
            if N_WARMUP_MM > 0:
                warm_sb = singles.tile([TPB, 512], _BF16)
                nc.vector.memset(warm_sb[:], 0.0)
                warm_ps = psum1p.tile([2 * D, 1024], _F32, tag="psum1")
                for _ in range(N_WARMUP_MM):
                    nc.tensor.matmul(
                        warm_ps[:, 0:512], warm_sb[:, 0:TPB], warm_sb[:, :],
                        start=True, stop=True,
                    )

            pseg_cur = None
            stash_act = {}    # m -> act1 handle awaiting L2
            stash_post = {}   # m -> psum2 tile awaiting act+mult
            stash = {}        # m -> msg handle awaiting segment acc

            def emit_l2(j):
                """L2 matmuls for batch j: per-tile stationary=act1 slice,
                moving=blockdiag(W2|G2)."""
                a = stash_act.pop(j)
                psum2 = psum2p.tile([TPB, L2B, 2 * D], _F32, tag="psum2")
                for tt in range(L2B):
                    sl = tt * TPB
                    nc.tensor.matmul(
                        psum2[:, tt, :], a[:, sl:sl + TPB], wg2_sb[:, :],
                        start=True, stop=not has_bias2,
                    )
                    if has_bias2:
                        nc.tensor.matmul(
                            psum2[:, tt, :], ones_sb[0:1, :],
                            b2g2_sb[0:1, :],
                            start=False, stop=True,
                        )
                stash_post[j] = psum2

            def emit_post(j):
                """Post-L2 stage: silu on ACT for h2pre, one fused
                custom-DVE op for ~2*sigmoid(g2pre), then the gate
                multiply (msg = 2x true message; the final atom add
                rescales by 0.5). Emitted one batch late so inputs are
                ready and the ACT queue never stalls."""
                psum2_j = stash_post.pop(j)
                h2 = l2p.tile([TPB, L2B, D], _BF16, tag="h2")
                nc.scalar.activation(h2[:], psum2_j[:, :, 0:D], AF.Silu)
                f2 = l2p.tile([TPB, L2B, D], _BF16, tag="f2")
                nc.vector._custom_dve(
                    GATE2_OP, out=f2[:], in0=psum2_j[:, :, D:2 * D],
                    s0=GATE_C0, s1=GATE_C1, imm2=GATE_C2,
                )
                msg = l2p.tile([TPB, L2B, D], _BF16, tag="msg")
                nc.vector.tensor_tensor(msg[:], h2[:], f2[:], mybir.AluOpType.mult)
                stash[j] = msg

            # blocks close in tile order; stream the output back to HBM in
            # chunks as soon as the last block of each chunk is done so the
            # final transfer isn't serialized after the last matmul
            out_edges = [
                (k * NBLK) // OUT_CHUNKS for k in range(1, OUT_CHUNKS + 1)
            ]

            def emit_seg(j):
                nonlocal pseg_cur
                msg_q = stash.pop(j)
                t0q = j * L2B
                # a batch's tiles live in one 16-batch s8 chunk; pick the
                # kept handle that covers this batch's columns
                want = t0q * BLK
                s8_sb, s8_c0 = next(
                    (sb, c0) for (sb, c0) in s8_chunk
                    if c0 <= want < c0 + CHUNK_B * 512
                )
                for tt in range(L2B):
                    t_glob = t0q + tt
                    b = int(tile_block[t_glob])
                    if first_of_block[t_glob]:
                        pseg_cur = psegp.tile([D, BLK], _F32, tag="pseg")
                    so = t_glob * BLK - s8_c0
                    nc.tensor.matmul(
                        pseg_cur[:, :], msg_q[:, tt, :],
                        s8_sb[:, so:so + BLK],
                        start=bool(first_of_block[t_glob]),
                        stop=bool(last_of_block[t_glob]),
                    )
                    if last_of_block[t_glob]:
                        # messages were doubled by the gate op; rescale here
                        nc.vector.affine_then_add(
                            out_sb[:, b * BLK:(b + 1) * BLK],
                            pseg_cur[:, :],
                            atom_sb[:, b * BLK:(b + 1) * BLK],
                            scale=0.5, bias=0.0,
                        )
                        if b + 1 in out_edges:
                            lo = out_edges[out_edges.index(b + 1) - 1] * BLK \
                                if out_edges.index(b + 1) > 0 else 0
                            nc.sync.dma_start(
                                out=out_d[:, lo:(b + 1) * BLK],
                                in_=out_sb[:, lo:(b + 1) * BLK],
                            )

            # s8 chunk bookkeeping: emit_seg(q) runs ~5 batches behind the
            # xt chunk loop, so the s8 chunk DMA is issued on the same
            # 16-batch cadence but the chunk consumed by emit_seg is the
            # PREVIOUS one for the first 5 batches of each chunk window.
            # Keep the last two chunk handles.
            s8_chunk = []  # [(tile, col0)], newest first

            for m2 in range(nb2):
                # ---- xt + s8 chunk DMA (every CHUNK_B batches); the first
                # chunk is split so compute starts after 512KB instead of
                # 2MB; s8 is only needed by emit_seg (runs ~5 batches late)
                if m2 % CHUNK_B == 0:
                    w = min(CHUNK_B, nb2 - m2) * 512
                    xt_sb = xtp.tile([2 * D, CHUNK_B * 512], _BF16, tag="xt")
                    if m2 == 0:
                        # first 512 cols = exactly batch 0 - compute can
                        # start after 128KB instead of 512KB
                        nc.sync.dma_start(
                            out=xt_sb[:, 0:512], in_=xt_d[:, 0:512]
                        )
                        nc.sync.dma_start(out=wg2_sb[:], in_=wg2_d[:, :])
                        nc.sync.dma_start(
                            out=xt_sb[:, 512:2048], in_=xt_d[:, 512:2048]
                        )
                        if has_bias1 or has_bias2:
                            nc.sync.dma_start(out=b1g1_sb[:], in_=b1g1_d[:, :])
                            nc.sync.dma_start(out=b2g2_sb[:], in_=b2g2_d[:, :])
                        ahi0 = NA_PAD // ATOM_CHUNKS
                        nc.sync.dma_start(
                            out=atom_sb[:, 0:ahi0], in_=atomT_d[:, 0:ahi0]
                        )
                        nc.sync.dma_start(
                            out=xt_sb[:, 2048:4096], in_=xt_d[:, 2048:4096]
                        )
                        nc.sync.dma_start(
                            out=xt_sb[:, 4096:w], in_=xt_d[:, 4096:w]
                        )
                    else:
                        nc.sync.dma_start(
                            out=xt_sb[:, :w],
                            in_=xt_d[:, m2 * 512: m2 * 512 + w],
                        )
                    s8_sb = s8p.tile([TPB, CHUNK_B * 512], _BF16, tag="s8")
                    c0 = m2 * L2B * BLK
                    ws = min(CHUNK_B, nb2 - m2) * 512
                    nc.sync.dma_start(
                        out=s8_sb[:, :ws], in_=s8_d[:, c0:c0 + ws]
                    )
                    s8_chunk.insert(0, (s8_sb, c0))
                    del s8_chunk[2:]

                # ---- L1: two K=64 row-packed MMs (PE rows 0-63 / 64-127
                # stream concurrently), one FD=1024 silu over both halves
                co = (m2 % CHUNK_B) * 512
                psum1 = psum1p.tile([2 * D, 1024], _F32, tag="psum1")
                nc.tensor.matmul(
                    psum1[:, 0:512], wg1_sb[0:D, :], xt_sb[0:D, co:co + 512],
                    start=True, stop=True,
                )
                nc.tensor.matmul(
                    psum1[:, 512:1024], wg1_sb[D:2 * D, :],
                    xt_sb[D:2 * D, co:co + 512],
                    start=True, stop=True,
                )
                act1 = actp.tile([2 * D, 1024], _BF16, tag="act1")
                if has_bias1:
                    nc.scalar.activation(
                        act1[:], psum1[:], AF.Silu, bias=b1g1_sb[:, 0:1],
                        scale=1.0,
                    )
                else:
                    nc.scalar.activation(act1[:], psum1[:], AF.Silu)
                stash_act[m2] = act1

                # post-L2 stage for batch m2-2, right after act1 in the
                # ACT queue (inputs two iterations old - ACT never stalls)
                if m2 >= 2:
                    emit_post(m2 - 2)

                # ---- L2 for batch m2-1 (act1 one iteration old, so the
                # act1->L2->h2 chain spans iterations instead of serializing
                # inside one period)
                if m2 >= 1:
                    emit_l2(m2 - 1)

                # ---- atom features arrive in chunks during the first
                # iterations instead of one serial 3.2MB DMA before compute
                if 1 <= m2 < ATOM_CHUNKS:
                    alo = (m2 * NA_PAD) // ATOM_CHUNKS
                    ahi = ((m2 + 1) * NA_PAD) // ATOM_CHUNKS
                    nc.sync.dma_start(
                        out=atom_sb[:, alo:ahi], in_=atomT_d[:, alo:ahi]
                    )

                # ---- segment accumulation for batch m2-3
                if m2 >= 3:
                    emit_seg(m2 - 3)

            emit_l2(nb2 - 1)
            emit_post(nb2 - 2)
            if nb2 >= 3:
                emit_seg(nb2 - 3)
            emit_post(nb2 - 1)
            emit_seg(nb2 - 2)
            emit_seg(nb2 - 1)

    # run_bass_kernel_spmd's serialization path skips the Bacc compile step
    # that fills in the 64-byte ISA payload of InstCustomDveAnt; without
    # this, walrus codegen fails with "ISA wrong length".
    mybir.codegen_inst_isa_subclasses(nc)
    return nc


# ----------------------------------------------------------------- kernel()
LAST_EXEC_NS = None
LAST_RESULT = None


def kernel(**inputs):
    atom_features = np.asarray(inputs["atom_features"], dtype=np.float32)
    bond_features = np.asarray(inputs["bond_features"], dtype=np.float32)
    bond_atom_indices = np.asarray(inputs["bond_atom_indices"])
    W1 = np.asarray(inputs["W1"], dtype=np.float32)
    W2 = np.asarray(inputs["W2"], dtype=np.float32)
    G1 = np.asarray(inputs["G1"], dtype=np.float32)
    G2 = np.asarray(inputs["G2"], dtype=np.float32)
    b1 = np.asarray(inputs["b1"], dtype=np.float32)
    b2 = np.asarray(inputs["b2"], dtype=np.float32)
    g1 = np.asarray(inputs["g1"], dtype=np.float32)
    g2 = np.asarray(inputs["g2"], dtype=np.float32)

    ntiles, tile_block, xt_list, s8_list, atomT_list = _plan_and_pack(
        atom_features, bond_features, bond_atom_indices
    )
    wg1, wg2, b1g1, b2g2 = _pack_weights(W1, G1, W2, G2, b1, g1, b2, g2)
    has_bias1 = not (np.all(b1 == 0.0) and np.all(g1 == 0.0))
    has_bias2 = not (np.all(b2 == 0.0) and np.all(g2 == 0.0))

    nc = _build_nc(ntiles, tile_block, has_bias1, has_bias2)

    in_maps = []
    for c in range(N_CORES):
        in_maps.append({
            "xt": xt_list[c],
            "s8": s8_list[c],
            "atomT": atomT_list[c],
            "wg1": wg1,
            "wg2": wg2,
            "b1g1": b1g1,
            "b2g2": b2g2,
        })

    import os as _os
    _trace = bool(int(_os.environ.get("KERNEL_TRACE", "0")))
    res = run_bass_kernel_spmd(nc, in_maps, core_ids=list(range(N_CORES)), trace=_trace)
    global LAST_EXEC_NS, LAST_RESULT
    LAST_EXEC_NS = res.exec_time_ns
    LAST_RESULT = res

    out = np.empty((N_ATOMS, D), dtype=np.float32)
    for c in range(N_CORES):
        out[c * NA_CORE:(c + 1) * NA_CORE] = res.results[c]["out"][:, :NA_CORE].T
    return out
